# revision 1
# baseline (speedup 1.0000x reference)
"""Trainium2 Bass kernel for nn_Correction_Module_dense.

Computation (bit-exact with the jax reference):
    grad   = x - roll(x, 1, axis=1)              # circular diff along neuron axis
    lower  = mean_grad - k*sqrt(var_grad)        # per-neuron, computed on host
    upper  = mean_grad + k*sqrt(var_grad)
    y      = x * (grad >= lower) * (grad <= upper)

Sharding: pure data parallel over the batch dim; 8 cores x [512, 8192] slabs.
Layout: batch rows -> partitions, neurons -> free axis (circular diff is a
free-dim offset AP).  lower/upper are broadcast once into [128, n] SBUF
tensors by log2-doubling SBUF->SBUF DMAs.

Raw-bass implementation (explicit semaphores): the toolchain's walrus codegen
allows only one inline sync-wait per compute instruction, which breaks
TileContext's packed waits for this dependency pattern; raw blocks emit
stand-alone wait_ge instructions instead.

Engine split per column-chunk:
    Pool (gpsimd): g = x - x_shift
    DVE (vector):  p = g >= lower; q = g <= upper; r = p*q (in place); y = r*x
    SP (sync):     all DMAs (loads, broadcast, stores)
"""

import numpy as np

import concourse.bass as bass
import concourse.mybir as mybir

B, N = 4096, 8192
N_CORES = 8
ROWS = B // N_CORES  # rows per core
P = 128


def build_nc(rows=ROWS, n=N, chunk=1024):
    nt = rows // P          # row tiles
    nch = n // chunk        # chunks per row tile
    f32 = mybir.dt.float32
    sub = mybir.AluOpType.subtract
    mul = mybir.AluOpType.mult
    is_ge = mybir.AluOpType.is_ge
    is_le = mybir.AluOpType.is_le

    XB = 2   # xt buffers
    YB = 4   # ym buffers
    GB = 2   # g buffers

    nc = bass.Bass()
    x = nc.dram_tensor("x", [rows, n], f32, kind="ExternalInput")
    low = nc.dram_tensor("low", [n], f32, kind="ExternalInput")
    up = nc.dram_tensor("up", [n], f32, kind="ExternalInput")
    y = nc.dram_tensor("y", [rows, n], f32, kind="ExternalOutput")

    from contextlib import ExitStack

    with ExitStack() as ctx:
        blow = ctx.enter_context(nc.sbuf_tensor("blow", [P, n], f32))
        bup = ctx.enter_context(nc.sbuf_tensor("bup", [P, n], f32))
        xt = [
            ctx.enter_context(nc.sbuf_tensor(f"xt{i}", [P, n], f32))
            for i in range(XB)
        ]
        g = [
            ctx.enter_context(nc.sbuf_tensor(f"g{i}", [P, chunk], f32))
            for i in range(GB)
        ]
        pm = [
            ctx.enter_context(nc.sbuf_tensor(f"pm{i}", [P, chunk], f32))
            for i in range(GB)
        ]
        qm = [
            ctx.enter_context(nc.sbuf_tensor(f"qm{i}", [P, chunk], f32))
            for i in range(GB)
        ]
        rm = [
            ctx.enter_context(nc.sbuf_tensor(f"rm{i}", [P, chunk], f32))
            for i in range(GB)
        ]
        ym = [
            ctx.enter_context(nc.sbuf_tensor(f"ym{i}", [P, chunk], f32))
            for i in range(YB)
        ]
        # One in-flight DMA per semaphore so sem-threshold waits are safe
        # under out-of-order DMA completion.
        LB = ctx.enter_context(nc.semaphore("LB"))  # broadcast chain (x16)
        Lb = [ctx.enter_context(nc.semaphore(f"Lb{i}")) for i in range(XB)]
        Sb = [ctx.enter_context(nc.semaphore(f"Sb{i}")) for i in range(YB)]
        PS = ctx.enter_context(nc.semaphore("PS"))  # pool g-chunk progress
        V = ctx.enter_context(nc.semaphore("V"))  # dve y-chunk progress
        block = ctx.enter_context(nc.Block())

        # 8 broadcast DMAs per bounds tensor: 1 load + 7 doublings
        n_bcast = 2 * 8
        l_bcast = 16 * n_bcast
        assert nch % YB == 0
        spt = nch // YB  # stores per ym buffer per row tile

        @block.sync
        def _(sync):
            lv = 0
            for vec, t in ((low, blow), (up, bup)):
                sync.dma_start(out=t[0:1, :], in_=vec[None, :]).then_inc(LB, 16)
                lv += 16
                pcnt = 1
                while pcnt < P:
                    sync.wait_ge(LB, lv)
                    sync.dma_start(
                        out=t[pcnt : 2 * pcnt, :], in_=t[0:pcnt, :]
                    ).then_inc(LB, 16)
                    lv += 16
                    pcnt *= 2
            for t in range(nt):
                if t >= XB:
                    # xt[t % XB] reusable once tile t-XB fully stored
                    for i in range(YB):
                        sync.wait_ge(Sb[i], 16 * spt * (t - XB + 1))
                sync.dma_start(
                    out=xt[t % XB][:], in_=x[t * P : (t + 1) * P, :]
                ).then_inc(Lb[t % XB], 16)
                for c in range(nch):
                    idx = t * nch + c
                    sync.wait_ge(V, idx + 1)
                    sync.dma_start(
                        out=y[t * P : (t + 1) * P, c * chunk : (c + 1) * chunk],
                        in_=ym[idx % YB][:],
                    ).then_inc(Sb[idx % YB], 16)

        @block.gpsimd
        def _(gpsimd):
            for t in range(nt):
                gpsimd.wait_ge(Lb[t % XB], 16 * (t // XB + 1))
                xb = xt[t % XB]
                for c in range(nch):
                    idx = t * nch + c
                    if idx >= GB:
                        gpsimd.wait_ge(V, idx - GB + 1)
                    gb = g[idx % GB]
                    c0 = c * chunk
                    if c == 0:
                        gpsimd.tensor_tensor(
                            gb[:, 1:chunk], xb[:, 1:chunk], xb[:, 0 : chunk - 1], sub
                        )
                        gpsimd.tensor_tensor(
                            gb[:, 0:1], xb[:, 0:1], xb[:, n - 1 : n], sub
                        ).then_inc(PS, 1)
                    else:
                        gpsimd.tensor_tensor(
                            gb[:], xb[:, c0 : c0 + chunk], xb[:, c0 - 1 : c0 + chunk - 1], sub
                        ).then_inc(PS, 1)

        @block.vector
        def _(vector):
            vector.wait_ge(LB, l_bcast)
            for t in range(nt):
                vector.wait_ge(Lb[t % XB], 16 * (t // XB + 1))
                xb = xt[t % XB]
                for c in range(nch):
                    idx = t * nch + c
                    c0 = c * chunk
                    gb = g[idx % GB]
                    pb = pm[idx % GB]
                    qb = qm[idx % GB]
                    rb = rm[idx % GB]
                    yb = ym[idx % YB]
                    vector.wait_ge(PS, idx + 1)
                    if idx >= YB:
                        vector.wait_ge(Sb[idx % YB], 16 * (idx // YB))
                    vector.tensor_tensor(pb[:], gb[:], blow[:, c0 : c0 + chunk], is_ge)
                    vector.tensor_tensor(qb[:], gb[:], bup[:, c0 : c0 + chunk], is_le)
                    vector.drain()
                    vector.tensor_tensor(rb[:], pb[:], qb[:], mul)
                    vector.drain()
                    vector.tensor_tensor(
                        yb[:], rb[:], xb[:, c0 : c0 + chunk], mul
                    ).then_inc(V, 1)

    return nc


def _host_bounds(mean_grad, var_grad, k):
    mg = np.asarray(mean_grad, dtype=np.float32)
    vg = np.asarray(var_grad, dtype=np.float32)
    kf = np.float32(k)
    std = np.sqrt(vg, dtype=np.float32)
    ks = (kf * std).astype(np.float32)
    lower = (mg - ks).astype(np.float32)
    upper = (mg + ks).astype(np.float32)
    return lower, upper


_NC_CACHE = {}


def kernel(output, mean_grad, var_grad, k):
    from concourse.bass_utils import run_bass_kernel_spmd

    x = np.ascontiguousarray(np.asarray(output, dtype=np.float32))
    assert x.shape == (B, N), x.shape
    lower, upper = _host_bounds(mean_grad, var_grad, k)

    if "nc" not in _NC_CACHE:
        _NC_CACHE["nc"] = build_nc()
    nc = _NC_CACHE["nc"]

    in_maps = [
        {"x": x[i * ROWS : (i + 1) * ROWS], "low": lower, "up": upper}
        for i in range(N_CORES)
    ]
    res = run_bass_kernel_spmd(nc, in_maps, core_ids=list(range(N_CORES)))
    return np.concatenate([res.results[i]["y"] for i in range(N_CORES)], axis=0)



# revision 3
# speedup vs baseline: 6.0648x; 6.0648x over previous
"""Trainium2 Bass kernel for nn_Correction_Module_dense.

Reference computation:
    grad  = x - roll(x, 1, axis=1)            # circular diff along neuron axis
    lower = mean_grad - k*sqrt(var_grad)      # per-neuron
    upper = mean_grad + k*sqrt(var_grad)
    y     = x * (lower <= grad) * (grad <= upper)

End-to-end wall time is dominated by the ~40 MB/s axon tunnel, so the
kernel is built around minimizing bytes on the wire:

  host:   q = rint(x * 127/max|x|)  (int8, 32 MB instead of 128 MB f32)
  device: g = q[i] - q[i-1]  (integer steps, exact in bf16)
          tight = L1 <= g <= U1   (margin +2 steps inside the band)
          loose = L0 <= g <= U0   (margin -2 steps outside the band)
          out   = packed uint16 per 8 neurons: low byte = tight bits,
                  high byte = loose bits  (8 MB total)
  host:   y = where(tight, x, 0); elements with loose & ~tight are within
          +-2 quantization steps of a boundary -> recompute exactly in f32.

Since |g_true/step - g_q| <= 1 + eps, tight => truly in-range and
!loose => truly out-of-range, so after the exact fixup of the uncertain
band the result is bit-identical to the f32 reference.

Sharding: pure data parallel over batch; 8 cores x [512, 8192].
Layout: batch rows -> partitions, neurons -> free axis.  Threshold
vectors are broadcast to 128 partitions once via log2-doubling DMAs.

The jitted shard_map executable is cached across calls (rebuilding it
per call costs a full retrace + PJRT compile); donated output buffers
are created on-device (never shipped over the tunnel).
"""

import numpy as np

import concourse.bass as bass
import concourse.mybir as mybir

B, N = 4096, 8192
N_CORES = 8
ROWS = B // N_CORES   # rows per core
P = 128
NT = ROWS // P        # row tiles per core
CH = 2048             # neuron chunk
NCH = N // CH
NPK = N // 8          # packed uint16s per row
MARGIN = 2.0          # uncertain band half-width, in quantization steps


def build_nc(rows=ROWS, n=N, chunk=CH):
    nt = rows // P
    nch = n // chunk
    npk_t = chunk // 8          # packed u16 per chunk
    grp = chunk // 8            # groups of 8 per chunk
    bf16 = mybir.dt.bfloat16
    f32 = mybir.dt.float32
    i8 = mybir.dt.int8
    u16 = mybir.dt.uint16
    sub = mybir.AluOpType.subtract
    mul = mybir.AluOpType.mult
    is_ge = mybir.AluOpType.is_ge
    is_le = mybir.AluOpType.is_le

    nc = bass.Bass()
    xq = nc.dram_tensor("xq", [rows, n], i8, kind="ExternalInput")
    # thr = [L1 | U1 | L0 | U0], each [n], integer-valued, |.| <= 255
    thr = nc.dram_tensor("thr", [4 * n], bf16, kind="ExternalInput")
    # wrow = 2^j pattern repeating every 16: [1,2,...,32768]*...
    wrow = nc.dram_tensor("wrow", [2 * chunk], bf16, kind="ExternalInput")
    out = nc.dram_tensor("out", [rows, n // 8], u16, kind="ExternalOutput")

    from contextlib import ExitStack

    with ExitStack() as ctx:
        THR = ctx.enter_context(nc.sbuf_tensor("THR", [P, 4 * n], bf16))
        WB = ctx.enter_context(nc.sbuf_tensor("WB", [P, 2 * chunk], bf16))
        XQ = [
            ctx.enter_context(nc.sbuf_tensor(f"XQ{t}", [P, n], i8))
            for t in range(nt)
        ]
        G = [
            ctx.enter_context(nc.sbuf_tensor(f"G{i}", [P, chunk], bf16))
            for i in range(2)
        ]
        A = ctx.enter_context(nc.sbuf_tensor("A", [P, chunk], bf16))
        Bb = ctx.enter_context(nc.sbuf_tensor("Bb", [P, chunk], bf16))
        A2 = ctx.enter_context(nc.sbuf_tensor("A2", [P, chunk], bf16))
        B2 = ctx.enter_context(nc.sbuf_tensor("B2", [P, chunk], bf16))
        TU = [
            ctx.enter_context(nc.sbuf_tensor(f"TU{i}", [P, 2 * chunk], bf16))
            for i in range(2)
        ]
        WM = [
            ctx.enter_context(nc.sbuf_tensor(f"WM{i}", [P, 2 * chunk], bf16))
            for i in range(2)
        ]
        PK = [
            ctx.enter_context(nc.sbuf_tensor(f"PK{i}", [P, npk_t], f32))
            for i in range(2)
        ]
        OUT = [
            ctx.enter_context(nc.sbuf_tensor(f"OUT{i}", [P, n // 8], u16))
            for i in range(2)
        ]

        LB = ctx.enter_context(nc.semaphore("LB"))   # broadcast chain
        LX = [ctx.enter_context(nc.semaphore(f"LX{t}")) for t in range(nt)]
        PS = ctx.enter_context(nc.semaphore("PS"))   # gpsimd chunk progress
        V = ctx.enter_context(nc.semaphore("V"))     # vector chunk progress
        C1 = ctx.enter_context(nc.semaphore("C1"))   # scalar copy progress
        SB = [ctx.enter_context(nc.semaphore(f"SB{i}")) for i in range(2)]
        block = ctx.enter_context(nc.Block())

        n_bcast = 2 * 8  # (1 load + 7 doublings) x 2 tensors
        l_bcast = 16 * n_bcast

        @block.sync
        def _(sync):
            lv = 0
            for vec, t in ((thr, THR), (wrow, WB)):
                sync.dma_start(out=t[0:1, :], in_=vec[None, :]).then_inc(LB, 16)
                lv += 16
                pcnt = 1
                while pcnt < P:
                    sync.wait_ge(LB, lv)
                    sync.dma_start(
                        out=t[pcnt : 2 * pcnt, :], in_=t[0:pcnt, :]
                    ).then_inc(LB, 16)
                    lv += 16
                    pcnt *= 2
            # all x tile loads issued upfront (XQ is nt-buffered)
            for t in range(nt):
                sync.dma_start(
                    out=XQ[t][:], in_=xq[t * P : (t + 1) * P, :]
                ).then_inc(LX[t], 16)
            # stores, one per row tile
            for t in range(nt):
                sync.wait_ge(C1, 4 * t + 4)
                sync.dma_start(
                    out=out[t * P : (t + 1) * P, :], in_=OUT[t % 2][:]
                ).then_inc(SB[t % 2], 16)

        @block.gpsimd
        def _(gpsimd):
            for t in range(nt):
                gpsimd.wait_ge(LX[t], 16)
                xb = XQ[t]
                for c in range(nch):
                    idx = t * nch + c
                    if idx >= 2:
                        gpsimd.wait_ge(V, idx - 1)
                    gb = G[idx % 2]
                    c0 = c * chunk
                    if c == 0:
                        gpsimd.tensor_tensor(
                            gb[:, 1:chunk], xb[:, 1:chunk], xb[:, 0 : chunk - 1], sub
                        )
                        gpsimd.tensor_tensor(
                            gb[:, 0:1], xb[:, 0:1], xb[:, n - 1 : n], sub
                        ).then_inc(PS, 1)
                    else:
                        gpsimd.tensor_tensor(
                            gb[:], xb[:, c0 : c0 + chunk],
                            xb[:, c0 - 1 : c0 + chunk - 1], sub
                        ).then_inc(PS, 1)

        @block.vector
        def _(vector):
            vector.wait_ge(LB, l_bcast)
            for t in range(nt):
                for c in range(nch):
                    idx = t * nch + c
                    c0 = c * chunk
                    gb = G[idx % 2]
                    tu = TU[idx % 2]
                    wm = WM[idx % 2]
                    pk = PK[idx % 2]
                    vector.wait_ge(PS, idx + 1)
                    if idx >= 2:
                        vector.wait_ge(C1, idx - 1)
                    vector.tensor_tensor(
                        A[:], gb[:], THR[:, 0 * n + c0 : 0 * n + c0 + chunk], is_ge
                    )
                    vector.tensor_tensor(
                        Bb[:], gb[:], THR[:, 1 * n + c0 : 1 * n + c0 + chunk], is_le
                    )
                    vector.tensor_tensor(
                        A2[:], gb[:], THR[:, 2 * n + c0 : 2 * n + c0 + chunk], is_ge
                    )
                    vector.tensor_tensor(
                        B2[:], gb[:], THR[:, 3 * n + c0 : 3 * n + c0 + chunk], is_le
                    )
                    vector.drain()
                    tu3 = tu[:].rearrange("p (g k) -> p g k", k=16)
                    vector.tensor_tensor(
                        tu3[:, :, 0:8],
                        A[:].rearrange("p (g k) -> p g k", k=8),
                        Bb[:].rearrange("p (g k) -> p g k", k=8),
                        mul,
                    )
                    vector.tensor_tensor(
                        tu3[:, :, 8:16],
                        A2[:].rearrange("p (g k) -> p g k", k=8),
                        B2[:].rearrange("p (g k) -> p g k", k=8),
                        mul,
                    )
                    vector.drain()
                    vector.tensor_tensor(wm[:], tu[:], WB[:], mul)
                    vector.drain()
                    vector.tensor_reduce(
                        pk[:],
                        wm[:].rearrange("p (g k) -> p g k", k=16),
                        mybir.AxisListType.X,
                        mybir.AluOpType.add,
                    ).then_inc(V, 1)

        @block.scalar
        def _(scalar):
            for t in range(nt):
                for c in range(nch):
                    idx = t * nch + c
                    scalar.wait_ge(V, idx + 1)
                    if c == 0 and t >= 2:
                        scalar.wait_ge(SB[t % 2], 16 * (t // 2))
                    scalar.copy(
                        OUT[t % 2][:, c * npk_t : (c + 1) * npk_t],
                        PK[idx % 2][:],
                    ).then_inc(C1, 1)

    return nc


_STATE = {}


def _get_runner():
    """Build (once) the cached jitted shard_map executable over 8 cores."""
    if "fn" in _STATE:
        return _STATE

    import jax
    import jax.numpy as jnp
    from jax.sharding import Mesh, PartitionSpec, NamedSharding
    from concourse import bass2jax

    try:
        from jax.experimental.shard_map import shard_map
    except ImportError:
        from jax.sharding import shard_map

    bass2jax.install_neuronx_cc_hook()

    nc = build_nc()
    assert nc.dbg_addr is None
    pid_name = nc.partition_id_tensor.name if nc.partition_id_tensor else None

    in_names = []
    out_names = []
    out_avals = []
    for alloc in nc.m.functions[0].allocations:
        if not isinstance(alloc, mybir.MemoryLocationSet):
            continue
        name = alloc.memorylocations[0].name
        if alloc.kind == "ExternalInput":
            if name != pid_name:
                in_names.append(name)
        elif alloc.kind == "ExternalOutput":
            out_names.append(name)
            out_avals.append(
                jax.core.ShapedArray(
                    tuple(alloc.tensor_shape), mybir.dt.np(alloc.dtype)
                )
            )
    assert in_names == ["xq", "thr", "wrow"], in_names
    assert out_names == ["out"], out_names
    all_in_names = tuple(in_names) + tuple(out_names)
    if pid_name is not None:
        all_in_names = all_in_names + (pid_name,)

    def _body(xq_a, thr_a, wrow_a, ybuf_a):
        operands = [xq_a, thr_a, wrow_a, ybuf_a]
        if pid_name is not None:
            operands.append(bass2jax.partition_id_tensor())
        outs = bass2jax._bass_exec_p.bind(
            *operands,
            out_avals=tuple(out_avals),
            in_names=all_in_names,
            out_names=tuple(out_names),
            lowering_input_output_aliases=(),
            sim_require_finite=True,
            sim_require_nnan=True,
            nc=nc,
        )
        return outs[0]

    devices = jax.devices()[:N_CORES]
    assert len(devices) == N_CORES
    mesh = Mesh(np.asarray(devices), ("core",))
    p_core = PartitionSpec("core")
    p_rep = PartitionSpec()
    fn = jax.jit(
        shard_map(
            _body,
            mesh=mesh,
            in_specs=(p_core, p_rep, p_rep, p_core),
            out_specs=p_core,
            check_rep=False,
        ),
        donate_argnums=(3,),
        keep_unused=True,
    )
    zeros_fn = jax.jit(
        lambda: jnp.zeros((B, N // 8), jnp.uint16),
        out_shardings=NamedSharding(mesh, p_core),
    )

    import ml_dtypes

    wrow_np = np.tile((2.0 ** np.arange(16)).astype(ml_dtypes.bfloat16), 2 * CH // 16)
    _STATE.update(
        fn=fn,
        zeros_fn=zeros_fn,
        mesh=mesh,
        sh_core=NamedSharding(mesh, p_core),
        sh_rep=NamedSharding(mesh, p_rep),
        w_dev=jax.device_put(wrow_np, NamedSharding(mesh, p_rep)),
        bf16=ml_dtypes.bfloat16,
        jax=jax,
    )
    return _STATE


def kernel(output, mean_grad, var_grad, k):
    st = _get_runner()
    jax = st["jax"]

    x = np.ascontiguousarray(np.asarray(output, dtype=np.float32))
    assert x.shape == (B, N), x.shape
    mg = np.asarray(mean_grad, dtype=np.float32)
    vg = np.asarray(var_grad, dtype=np.float32)
    kf = np.float32(k)

    # f32 bounds, bit-matching the reference
    std = np.sqrt(vg, dtype=np.float32)
    ks = (kf * std).astype(np.float32)
    lower = (mg - ks).astype(np.float32)
    upper = (mg + ks).astype(np.float32)

    # quantize x to int8 steps
    maxabs = float(np.abs(x).max())
    if maxabs == 0.0:
        maxabs = 1.0
    inv = np.float32(127.0 / maxabs)
    q = np.rint(x * inv).astype(np.int8)

    # integer thresholds in step units (margin 2 steps; |g| <= 254)
    los = lower.astype(np.float64) * (127.0 / maxabs)
    ups = upper.astype(np.float64) * (127.0 / maxabs)
    L1 = np.clip(np.ceil(los) + MARGIN, -255, 255)
    U1 = np.clip(np.floor(ups) - MARGIN, -255, 255)
    L0 = np.clip(np.ceil(los) - MARGIN, -255, 255)
    U0 = np.clip(np.floor(ups) + MARGIN, -255, 255)
    thr_np = np.concatenate([L1, U1, L0, U0]).astype(st["bf16"])

    # transfers (x_q is the big one: 32 MB)
    q_dev = jax.device_put(q, st["sh_core"])
    thr_dev = jax.device_put(thr_np, st["sh_rep"])
    ybuf = st["zeros_fn"]()
    out_dev = st["fn"](q_dev, thr_dev, st["w_dev"], ybuf)
    pk = np.asarray(out_dev)  # (B, N//8) uint16

    # decode: low byte = tight bits, high byte = loose bits (little endian)
    byte_view = pk.view(np.uint8).reshape(B, N // 8, 2)
    tight = np.unpackbits(byte_view[:, :, 0], axis=1, bitorder="little")
    loose = np.unpackbits(byte_view[:, :, 1], axis=1, bitorder="little")

    y = np.where(tight.astype(bool), x, np.float32(0.0))

    # exact fixup of the uncertain band
    unc = loose & (1 - tight)
    idx = np.flatnonzero(unc)
    if idx.size:
        xr = x.ravel()
        cols = idx & (N - 1)
        prev = idx - 1 + ((cols == 0).astype(np.int64) << 13)
        g_ex = xr[idx] - xr[prev]
        keep = (g_ex >= lower[cols]) & (g_ex <= upper[cols])
        y.ravel()[idx] = np.where(keep, xr[idx], np.float32(0.0))
    return y


# revision 5
# speedup vs baseline: 8.2343x; 1.3577x over previous
"""Trainium2 Bass kernel for nn_Correction_Module_dense.

Reference computation:
    grad  = x - roll(x, 1, axis=1)            # circular diff along neuron axis
    lower = mean_grad - k*sqrt(var_grad)      # per-neuron
    upper = mean_grad + k*sqrt(var_grad)
    y     = x * (lower <= grad) * (grad <= upper)

End-to-end wall time is dominated by the ~40 MB/s axon tunnel, so the
kernel is built around minimizing bytes on the wire:

  host:   q = rint(x * 127/max|x|)  (int8, 32 MB instead of 128 MB f32)
  device: g = q[i] - q[i-1]  (integer steps, exact in bf16)
          tight = L1 <= g <= U1   (margin +2 steps inside the band)
          loose = L0 <= g <= U0   (margin -2 steps outside the band)
          out   = packed uint16 per 8 neurons: low byte = tight bits,
                  high byte = loose bits  (8 MB total)
  host:   y = where(tight, x, 0); elements with loose & ~tight are within
          +-2 quantization steps of a boundary -> recompute exactly in f32.

Since |g_true/step - g_q| <= 1 + eps, tight => truly in-range and
!loose => truly out-of-range, so after the exact fixup of the uncertain
band the result is bit-identical to the f32 reference.

Sharding: pure data parallel over batch; 8 cores x [512, 8192].
Layout: batch rows -> partitions, neurons -> free axis.  Threshold
vectors are broadcast to 128 partitions once via log2-doubling DMAs.

The jitted shard_map executable is cached across calls (rebuilding it
per call costs a full retrace + PJRT compile); donated output buffers
are created on-device (never shipped over the tunnel).
"""

import numpy as np

import concourse.bass as bass
import concourse.mybir as mybir

B, N = 4096, 8192
N_CORES = 8
ROWS = B // N_CORES   # rows per core
P = 128
NT = ROWS // P        # row tiles per core
CH = 2048             # neuron chunk
NCH = N // CH
NPK = N // 8          # packed uint16s per row
MARGIN = 2.0          # uncertain band half-width, in quantization steps


def build_nc(rows=ROWS, n=N, chunk=CH):
    nt = rows // P
    nch = n // chunk
    npk_t = chunk // 8          # packed u16 per chunk
    grp = chunk // 8            # groups of 8 per chunk
    bf16 = mybir.dt.bfloat16
    f32 = mybir.dt.float32
    i8 = mybir.dt.int8
    u16 = mybir.dt.uint16
    sub = mybir.AluOpType.subtract
    mul = mybir.AluOpType.mult
    is_ge = mybir.AluOpType.is_ge
    is_le = mybir.AluOpType.is_le

    nc = bass.Bass()
    xq = nc.dram_tensor("xq", [rows, n], i8, kind="ExternalInput")
    # thr = [L1 | U1 | L0 | U0], each [n], integer-valued, |.| <= 255
    thr = nc.dram_tensor("thr", [4 * n], bf16, kind="ExternalInput")
    # wrow = 2^j pattern repeating every 16: [1,2,...,32768]*...
    wrow = nc.dram_tensor("wrow", [2 * chunk], bf16, kind="ExternalInput")
    out = nc.dram_tensor("out", [rows, n // 8], u16, kind="ExternalOutput")

    from contextlib import ExitStack

    with ExitStack() as ctx:
        THR = ctx.enter_context(nc.sbuf_tensor("THR", [P, 4 * n], bf16))
        WB = ctx.enter_context(nc.sbuf_tensor("WB", [P, 2 * chunk], bf16))
        XQ = [
            ctx.enter_context(nc.sbuf_tensor(f"XQ{t}", [P, n], i8))
            for t in range(nt)
        ]
        G = [
            ctx.enter_context(nc.sbuf_tensor(f"G{i}", [P, chunk], bf16))
            for i in range(2)
        ]
        A = ctx.enter_context(nc.sbuf_tensor("A", [P, chunk], bf16))
        Bb = ctx.enter_context(nc.sbuf_tensor("Bb", [P, chunk], bf16))
        A2 = ctx.enter_context(nc.sbuf_tensor("A2", [P, chunk], bf16))
        B2 = ctx.enter_context(nc.sbuf_tensor("B2", [P, chunk], bf16))
        TU = [
            ctx.enter_context(nc.sbuf_tensor(f"TU{i}", [P, 2 * chunk], bf16))
            for i in range(2)
        ]
        WM = [
            ctx.enter_context(nc.sbuf_tensor(f"WM{i}", [P, 2 * chunk], bf16))
            for i in range(2)
        ]
        PK = [
            ctx.enter_context(nc.sbuf_tensor(f"PK{i}", [P, npk_t], f32))
            for i in range(2)
        ]
        OUT = [
            ctx.enter_context(nc.sbuf_tensor(f"OUT{i}", [P, n // 8], u16))
            for i in range(2)
        ]

        LB = ctx.enter_context(nc.semaphore("LB"))   # broadcast chain
        LX = [ctx.enter_context(nc.semaphore(f"LX{t}")) for t in range(nt)]
        PS = ctx.enter_context(nc.semaphore("PS"))   # gpsimd chunk progress
        V = ctx.enter_context(nc.semaphore("V"))     # vector chunk progress
        C1 = ctx.enter_context(nc.semaphore("C1"))   # scalar copy progress
        SB = [ctx.enter_context(nc.semaphore(f"SB{i}")) for i in range(2)]
        block = ctx.enter_context(nc.Block())

        n_bcast = 2 * 8  # (1 load + 7 doublings) x 2 tensors
        l_bcast = 16 * n_bcast

        @block.sync
        def _(sync):
            lv = 0
            for vec, t in ((thr, THR), (wrow, WB)):
                sync.dma_start(out=t[0:1, :], in_=vec[None, :]).then_inc(LB, 16)
                lv += 16
                pcnt = 1
                while pcnt < P:
                    sync.wait_ge(LB, lv)
                    sync.dma_start(
                        out=t[pcnt : 2 * pcnt, :], in_=t[0:pcnt, :]
                    ).then_inc(LB, 16)
                    lv += 16
                    pcnt *= 2
            # all x tile loads issued upfront (XQ is nt-buffered)
            for t in range(nt):
                sync.dma_start(
                    out=XQ[t][:], in_=xq[t * P : (t + 1) * P, :]
                ).then_inc(LX[t], 16)
            # stores, one per row tile
            for t in range(nt):
                sync.wait_ge(C1, 4 * t + 4)
                sync.dma_start(
                    out=out[t * P : (t + 1) * P, :], in_=OUT[t % 2][:]
                ).then_inc(SB[t % 2], 16)

        @block.gpsimd
        def _(gpsimd):
            for t in range(nt):
                gpsimd.wait_ge(LX[t], 16)
                xb = XQ[t]
                for c in range(nch):
                    idx = t * nch + c
                    if idx >= 2:
                        gpsimd.wait_ge(V, idx - 1)
                    gb = G[idx % 2]
                    c0 = c * chunk
                    if c == 0:
                        gpsimd.tensor_tensor(
                            gb[:, 1:chunk], xb[:, 1:chunk], xb[:, 0 : chunk - 1], sub
                        )
                        gpsimd.tensor_tensor(
                            gb[:, 0:1], xb[:, 0:1], xb[:, n - 1 : n], sub
                        ).then_inc(PS, 1)
                    else:
                        gpsimd.tensor_tensor(
                            gb[:], xb[:, c0 : c0 + chunk],
                            xb[:, c0 - 1 : c0 + chunk - 1], sub
                        ).then_inc(PS, 1)

        @block.vector
        def _(vector):
            vector.wait_ge(LB, l_bcast)
            for t in range(nt):
                for c in range(nch):
                    idx = t * nch + c
                    c0 = c * chunk
                    gb = G[idx % 2]
                    tu = TU[idx % 2]
                    wm = WM[idx % 2]
                    pk = PK[idx % 2]
                    vector.wait_ge(PS, idx + 1)
                    if idx >= 2:
                        vector.wait_ge(C1, idx - 1)
                    vector.tensor_tensor(
                        A[:], gb[:], THR[:, 0 * n + c0 : 0 * n + c0 + chunk], is_ge
                    )
                    vector.tensor_tensor(
                        Bb[:], gb[:], THR[:, 1 * n + c0 : 1 * n + c0 + chunk], is_le
                    )
                    vector.tensor_tensor(
                        A2[:], gb[:], THR[:, 2 * n + c0 : 2 * n + c0 + chunk], is_ge
                    )
                    vector.tensor_tensor(
                        B2[:], gb[:], THR[:, 3 * n + c0 : 3 * n + c0 + chunk], is_le
                    )
                    vector.drain()
                    tu3 = tu[:].rearrange("p (g k) -> p g k", k=16)
                    vector.tensor_tensor(
                        tu3[:, :, 0:8],
                        A[:].rearrange("p (g k) -> p g k", k=8),
                        Bb[:].rearrange("p (g k) -> p g k", k=8),
                        mul,
                    )
                    vector.tensor_tensor(
                        tu3[:, :, 8:16],
                        A2[:].rearrange("p (g k) -> p g k", k=8),
                        B2[:].rearrange("p (g k) -> p g k", k=8),
                        mul,
                    )
                    vector.drain()
                    vector.tensor_tensor(wm[:], tu[:], WB[:], mul)
                    vector.drain()
                    vector.tensor_reduce(
                        pk[:],
                        wm[:].rearrange("p (g k) -> p g k", k=16),
                        mybir.AxisListType.X,
                        mybir.AluOpType.add,
                    ).then_inc(V, 1)

        @block.scalar
        def _(scalar):
            for t in range(nt):
                for c in range(nch):
                    idx = t * nch + c
                    scalar.wait_ge(V, idx + 1)
                    if c == 0 and t >= 2:
                        scalar.wait_ge(SB[t % 2], 16 * (t // 2))
                    scalar.copy(
                        OUT[t % 2][:, c * npk_t : (c + 1) * npk_t],
                        PK[idx % 2][:],
                    ).then_inc(C1, 1)

    return nc


_STATE = {}


def _get_runner():
    """Build (once) the cached jitted shard_map executable over 8 cores."""
    if "fn" in _STATE:
        return _STATE

    import jax
    import jax.numpy as jnp
    from jax.sharding import Mesh, PartitionSpec, NamedSharding
    from concourse import bass2jax

    try:
        from jax.experimental.shard_map import shard_map
    except ImportError:
        from jax.sharding import shard_map

    bass2jax.install_neuronx_cc_hook()

    nc = build_nc()
    assert nc.dbg_addr is None
    pid_name = nc.partition_id_tensor.name if nc.partition_id_tensor else None

    in_names = []
    out_names = []
    out_avals = []
    for alloc in nc.m.functions[0].allocations:
        if not isinstance(alloc, mybir.MemoryLocationSet):
            continue
        name = alloc.memorylocations[0].name
        if alloc.kind == "ExternalInput":
            if name != pid_name:
                in_names.append(name)
        elif alloc.kind == "ExternalOutput":
            out_names.append(name)
            out_avals.append(
                jax.core.ShapedArray(
                    tuple(alloc.tensor_shape), mybir.dt.np(alloc.dtype)
                )
            )
    assert in_names == ["xq", "thr", "wrow"], in_names
    assert out_names == ["out"], out_names
    all_in_names = tuple(in_names) + tuple(out_names)
    if pid_name is not None:
        all_in_names = all_in_names + (pid_name,)

    def _body(xq_a, thr_a, wrow_a, ybuf_a):
        operands = [xq_a, thr_a, wrow_a, ybuf_a]
        if pid_name is not None:
            operands.append(bass2jax.partition_id_tensor())
        outs = bass2jax._bass_exec_p.bind(
            *operands,
            out_avals=tuple(out_avals),
            in_names=all_in_names,
            out_names=tuple(out_names),
            lowering_input_output_aliases=(),
            sim_require_finite=True,
            sim_require_nnan=True,
            nc=nc,
        )
        return outs[0]

    devices = jax.devices()[:N_CORES]
    assert len(devices) == N_CORES
    mesh = Mesh(np.asarray(devices), ("core",))
    p_core = PartitionSpec("core")
    fn = jax.jit(
        shard_map(
            _body,
            mesh=mesh,
            in_specs=(p_core, p_core, p_core, p_core),
            out_specs=p_core,
            check_rep=False,
        ),
        donate_argnums=(3,),
        keep_unused=True,
    )
    zeros_fn = jax.jit(
        lambda: jnp.zeros((B, N // 8), jnp.uint16),
        out_shardings=NamedSharding(mesh, p_core),
    )

    import ml_dtypes

    wrow_np = np.tile(
        (2.0 ** np.arange(16)).astype(ml_dtypes.bfloat16), N_CORES * 2 * CH // 16
    )
    _STATE.update(
        fn=fn,
        zeros_fn=zeros_fn,
        mesh=mesh,
        sh_core=NamedSharding(mesh, p_core),
        w_dev=jax.device_put(wrow_np, NamedSharding(mesh, p_core)),
        bf16=ml_dtypes.bfloat16,
        jax=jax,
        qbuf=np.empty((B, N), np.float32),
        q8=np.empty((B, N), np.int8),
    )
    return _STATE


def kernel(output, mean_grad, var_grad, k):
    st = _get_runner()
    jax = st["jax"]

    x = np.ascontiguousarray(np.asarray(output, dtype=np.float32))
    assert x.shape == (B, N), x.shape
    mg = np.asarray(mean_grad, dtype=np.float32)
    vg = np.asarray(var_grad, dtype=np.float32)
    kf = np.float32(k)

    # f32 bounds, bit-matching the reference
    std = np.sqrt(vg, dtype=np.float32)
    ks = (kf * std).astype(np.float32)
    lower = (mg - ks).astype(np.float32)
    upper = (mg + ks).astype(np.float32)

    # quantize x to int8 steps (in-place via cached scratch buffers)
    maxabs = max(float(x.max()), -float(x.min()))
    if maxabs == 0.0:
        maxabs = 1.0
    inv = np.float32(127.0 / maxabs)
    buf, q8 = st["qbuf"], st["q8"]
    np.multiply(x, inv, out=buf)
    np.rint(buf, out=buf)
    np.copyto(q8, buf, casting="unsafe")

    # integer thresholds in step units (margin 2 steps; |g| <= 254)
    los = lower.astype(np.float64) * (127.0 / maxabs)
    ups = upper.astype(np.float64) * (127.0 / maxabs)
    L1 = np.clip(np.ceil(los) + MARGIN, -255, 255)
    U1 = np.clip(np.floor(ups) - MARGIN, -255, 255)
    L0 = np.clip(np.ceil(los) - MARGIN, -255, 255)
    U0 = np.clip(np.floor(ups) + MARGIN, -255, 255)
    thr_np = np.tile(
        np.concatenate([L1, U1, L0, U0]).astype(st["bf16"]), N_CORES
    )

    # transfers (x_q is the big one: 32 MB)
    q_dev = jax.device_put(q8, st["sh_core"])
    thr_dev = jax.device_put(thr_np, st["sh_core"])
    ybuf = _STATE.pop("ybuf_next", None)
    if ybuf is None:
        ybuf = st["zeros_fn"]()
    out_dev = st["fn"](q_dev, thr_dev, st["w_dev"], ybuf)
    # pre-create the next call's donated output buffer while fetching
    _STATE["ybuf_next"] = st["zeros_fn"]()
    pk = np.asarray(out_dev)  # (B, N//8) uint16

    # decode: low byte = tight bits, high byte = loose bits (little endian)
    byte_view = pk.view(np.uint8).reshape(B, N // 8, 2)
    t_bytes = np.ascontiguousarray(byte_view[:, :, 0])
    tight = np.unpackbits(t_bytes, axis=1, bitorder="little")
    y = np.multiply(x, tight)

    # exact fixup of the uncertain band (loose & ~tight), sparse decode
    u_bytes = (byte_view[:, :, 1] & ~t_bytes).ravel()
    nzb = np.flatnonzero(u_bytes)
    if nzb.size:
        bits = np.unpackbits(u_bytes[nzb], bitorder="little")
        pos = np.flatnonzero(bits)
        idx = (nzb[pos >> 3] << 3) + (pos & 7)
        xr = x.ravel()
        cols = idx & (N - 1)
        prev = idx - 1 + ((cols == 0).astype(np.int64) << 13)
        g_ex = xr[idx] - xr[prev]
        keep = (g_ex >= lower[cols]) & (g_ex <= upper[cols])
        y.ravel()[idx] = np.where(keep, xr[idx], np.float32(0.0))
    return y


# revision 8
# speedup vs baseline: 8.4375x; 1.0247x over previous
"""Trainium2 Bass kernel for nn_Correction_Module_dense.

Reference computation:
    grad  = x - roll(x, 1, axis=1)            # circular diff along neuron axis
    lower = mean_grad - k*sqrt(var_grad)      # per-neuron
    upper = mean_grad + k*sqrt(var_grad)
    y     = x * (lower <= grad) * (grad <= upper)

End-to-end wall time is dominated by the ~40 MB/s axon tunnel, so the
kernel is built around minimizing bytes on the wire:

  host:   q = rint(x * 127/max|x|)  (int8, 32 MB instead of 128 MB f32)
  device: g = q[i] - q[i-1]  (integer steps, exact in bf16)
          tight = L1 <= g <= U1   (margin +2 steps inside the band)
          loose = L0 <= g <= U0   (margin -2 steps outside the band)
          out   = packed uint16 per 8 neurons: low byte = tight bits,
                  high byte = loose bits  (8 MB total)
  host:   y = where(tight, x, 0); elements with loose & ~tight are within
          +-2 quantization steps of a boundary -> recompute exactly in f32.

Since |g_true/step - g_q| <= 1 + eps, tight => truly in-range and
!loose => truly out-of-range, so after the exact fixup of the uncertain
band the result is bit-identical to the f32 reference.

Sharding: pure data parallel over batch; 8 cores x [512, 8192].
Layout: batch rows -> partitions, neurons -> free axis.  Threshold
vectors are broadcast to 128 partitions once via log2-doubling DMAs.

The jitted shard_map executable is cached across calls (rebuilding it
per call costs a full retrace + PJRT compile); donated output buffers
are created on-device (never shipped over the tunnel).
"""

import numpy as np

import concourse.bass as bass
import concourse.mybir as mybir

B, N = 4096, 8192
N_CORES = 8
ROWS = B // N_CORES   # rows per core
P = 128
NT = ROWS // P        # row tiles per core
CH = 2048             # neuron chunk
NCH = N // CH
NPK = N // 8          # packed uint16s per row
MARGIN = 2.0          # uncertain band half-width, in quantization steps


def build_nc(rows=ROWS, n=N, chunk=CH):
    nt = rows // P
    nch = n // chunk
    npk_t = chunk // 8          # packed u16 per chunk
    grp = chunk // 8            # groups of 8 per chunk
    bf16 = mybir.dt.bfloat16
    f32 = mybir.dt.float32
    i8 = mybir.dt.int8
    u16 = mybir.dt.uint16
    sub = mybir.AluOpType.subtract
    mul = mybir.AluOpType.mult
    is_ge = mybir.AluOpType.is_ge
    is_le = mybir.AluOpType.is_le

    nc = bass.Bass()
    xq = nc.dram_tensor("xq", [rows, n], i8, kind="ExternalInput")
    # thr = [L1 | U1 | L0 | U0], each [n], integer-valued, |.| <= 255
    thr = nc.dram_tensor("thr", [4 * n], bf16, kind="ExternalInput")
    # wrow = 2^j pattern repeating every 16: [1,2,...,32768]*...
    wrow = nc.dram_tensor("wrow", [2 * chunk], bf16, kind="ExternalInput")
    out = nc.dram_tensor("out", [rows, n // 8], u16, kind="ExternalOutput")

    from contextlib import ExitStack

    with ExitStack() as ctx:
        THR = ctx.enter_context(nc.sbuf_tensor("THR", [P, 4 * n], bf16))
        WB = ctx.enter_context(nc.sbuf_tensor("WB", [P, 2 * chunk], bf16))
        XQ = [
            ctx.enter_context(nc.sbuf_tensor(f"XQ{t}", [P, n], i8))
            for t in range(nt)
        ]
        G = [
            ctx.enter_context(nc.sbuf_tensor(f"G{i}", [P, chunk], bf16))
            for i in range(2)
        ]
        A = ctx.enter_context(nc.sbuf_tensor("A", [P, chunk], bf16))
        Bb = ctx.enter_context(nc.sbuf_tensor("Bb", [P, chunk], bf16))
        A2 = ctx.enter_context(nc.sbuf_tensor("A2", [P, chunk], bf16))
        B2 = ctx.enter_context(nc.sbuf_tensor("B2", [P, chunk], bf16))
        TU = [
            ctx.enter_context(nc.sbuf_tensor(f"TU{i}", [P, 2 * chunk], bf16))
            for i in range(2)
        ]
        WM = [
            ctx.enter_context(nc.sbuf_tensor(f"WM{i}", [P, 2 * chunk], bf16))
            for i in range(2)
        ]
        PK = [
            ctx.enter_context(nc.sbuf_tensor(f"PK{i}", [P, npk_t], f32))
            for i in range(2)
        ]
        OUT = [
            ctx.enter_context(nc.sbuf_tensor(f"OUT{i}", [P, n // 8], u16))
            for i in range(2)
        ]

        LB = ctx.enter_context(nc.semaphore("LB"))   # broadcast chain
        LX = [ctx.enter_context(nc.semaphore(f"LX{t}")) for t in range(nt)]
        PS = ctx.enter_context(nc.semaphore("PS"))   # gpsimd chunk progress
        V = ctx.enter_context(nc.semaphore("V"))     # vector chunk progress
        C1 = ctx.enter_context(nc.semaphore("C1"))   # scalar copy progress
        SB = [ctx.enter_context(nc.semaphore(f"SB{i}")) for i in range(2)]
        block = ctx.enter_context(nc.Block())

        n_bcast = 2 * 8  # (1 load + 7 doublings) x 2 tensors
        l_bcast = 16 * n_bcast

        @block.sync
        def _(sync):
            lv = 0
            for vec, t in ((thr, THR), (wrow, WB)):
                sync.dma_start(out=t[0:1, :], in_=vec[None, :]).then_inc(LB, 16)
                lv += 16
                pcnt = 1
                while pcnt < P:
                    sync.wait_ge(LB, lv)
                    sync.dma_start(
                        out=t[pcnt : 2 * pcnt, :], in_=t[0:pcnt, :]
                    ).then_inc(LB, 16)
                    lv += 16
                    pcnt *= 2
            # all x tile loads issued upfront (XQ is nt-buffered)
            for t in range(nt):
                sync.dma_start(
                    out=XQ[t][:], in_=xq[t * P : (t + 1) * P, :]
                ).then_inc(LX[t], 16)
            # stores, one per row tile
            for t in range(nt):
                sync.wait_ge(C1, 4 * t + 4)
                sync.dma_start(
                    out=out[t * P : (t + 1) * P, :], in_=OUT[t % 2][:]
                ).then_inc(SB[t % 2], 16)

        @block.gpsimd
        def _(gpsimd):
            for t in range(nt):
                gpsimd.wait_ge(LX[t], 16)
                xb = XQ[t]
                for c in range(nch):
                    idx = t * nch + c
                    if idx >= 2:
                        gpsimd.wait_ge(V, idx - 1)
                    gb = G[idx % 2]
                    c0 = c * chunk
                    if c == 0:
                        gpsimd.tensor_tensor(
                            gb[:, 1:chunk], xb[:, 1:chunk], xb[:, 0 : chunk - 1], sub
                        )
                        gpsimd.tensor_tensor(
                            gb[:, 0:1], xb[:, 0:1], xb[:, n - 1 : n], sub
                        ).then_inc(PS, 1)
                    else:
                        gpsimd.tensor_tensor(
                            gb[:], xb[:, c0 : c0 + chunk],
                            xb[:, c0 - 1 : c0 + chunk - 1], sub
                        ).then_inc(PS, 1)

        @block.vector
        def _(vector):
            vector.wait_ge(LB, l_bcast)
            for t in range(nt):
                for c in range(nch):
                    idx = t * nch + c
                    c0 = c * chunk
                    gb = G[idx % 2]
                    tu = TU[idx % 2]
                    wm = WM[idx % 2]
                    pk = PK[idx % 2]
                    vector.wait_ge(PS, idx + 1)
                    if idx >= 2:
                        vector.wait_ge(C1, idx - 1)
                    vector.tensor_tensor(
                        A[:], gb[:], THR[:, 0 * n + c0 : 0 * n + c0 + chunk], is_ge
                    )
                    vector.tensor_tensor(
                        Bb[:], gb[:], THR[:, 1 * n + c0 : 1 * n + c0 + chunk], is_le
                    )
                    vector.tensor_tensor(
                        A2[:], gb[:], THR[:, 2 * n + c0 : 2 * n + c0 + chunk], is_ge
                    )
                    vector.tensor_tensor(
                        B2[:], gb[:], THR[:, 3 * n + c0 : 3 * n + c0 + chunk], is_le
                    )
                    vector.drain()
                    tu3 = tu[:].rearrange("p (g k) -> p g k", k=16)
                    vector.tensor_tensor(
                        tu3[:, :, 0:8],
                        A[:].rearrange("p (g k) -> p g k", k=8),
                        Bb[:].rearrange("p (g k) -> p g k", k=8),
                        mul,
                    )
                    vector.tensor_tensor(
                        tu3[:, :, 8:16],
                        A2[:].rearrange("p (g k) -> p g k", k=8),
                        B2[:].rearrange("p (g k) -> p g k", k=8),
                        mul,
                    )
                    vector.drain()
                    vector.tensor_tensor(wm[:], tu[:], WB[:], mul)
                    vector.drain()
                    vector.tensor_reduce(
                        pk[:],
                        wm[:].rearrange("p (g k) -> p g k", k=16),
                        mybir.AxisListType.X,
                        mybir.AluOpType.add,
                    ).then_inc(V, 1)

        @block.scalar
        def _(scalar):
            for t in range(nt):
                for c in range(nch):
                    idx = t * nch + c
                    scalar.wait_ge(V, idx + 1)
                    if c == 0 and t >= 2:
                        scalar.wait_ge(SB[t % 2], 16 * (t // 2))
                    scalar.copy(
                        OUT[t % 2][:, c * npk_t : (c + 1) * npk_t],
                        PK[idx % 2][:],
                    ).then_inc(C1, 1)

    return nc


_STATE = {}
SPLIT = 2                      # pipeline stages (device groups)
GSIZE = N_CORES // SPLIT       # cores per group
GROWS = B // SPLIT             # batch rows per group


def _get_runner():
    """Build (once) the cached jitted shard_map executables, one per
    device group (the batch is pipelined across groups so host work
    overlaps the ~40 MB/s tunnel transfers)."""
    if "groups" in _STATE:
        return _STATE

    import jax
    import jax.numpy as jnp
    from jax.sharding import Mesh, PartitionSpec, NamedSharding
    from concourse import bass2jax

    try:
        from jax.experimental.shard_map import shard_map
    except ImportError:
        from jax.sharding import shard_map

    bass2jax.install_neuronx_cc_hook()

    nc = build_nc()
    assert nc.dbg_addr is None
    pid_name = nc.partition_id_tensor.name if nc.partition_id_tensor else None

    in_names = []
    out_names = []
    out_avals = []
    for alloc in nc.m.functions[0].allocations:
        if not isinstance(alloc, mybir.MemoryLocationSet):
            continue
        name = alloc.memorylocations[0].name
        if alloc.kind == "ExternalInput":
            if name != pid_name:
                in_names.append(name)
        elif alloc.kind == "ExternalOutput":
            out_names.append(name)
            out_avals.append(
                jax.core.ShapedArray(
                    tuple(alloc.tensor_shape), mybir.dt.np(alloc.dtype)
                )
            )
    assert in_names == ["xq", "thr", "wrow"], in_names
    assert out_names == ["out"], out_names
    all_in_names = tuple(in_names) + tuple(out_names)
    if pid_name is not None:
        all_in_names = all_in_names + (pid_name,)

    def _body(xq_a, thr_a, wrow_a, ybuf_a):
        operands = [xq_a, thr_a, wrow_a, ybuf_a]
        if pid_name is not None:
            operands.append(bass2jax.partition_id_tensor())
        outs = bass2jax._bass_exec_p.bind(
            *operands,
            out_avals=tuple(out_avals),
            in_names=all_in_names,
            out_names=tuple(out_names),
            lowering_input_output_aliases=(),
            sim_require_finite=True,
            sim_require_nnan=True,
            nc=nc,
        )
        return outs[0]

    devices = jax.devices()[:N_CORES]
    assert len(devices) == N_CORES
    p_core = PartitionSpec("core")

    import ml_dtypes

    wrow_np = np.tile(
        (2.0 ** np.arange(16)).astype(ml_dtypes.bfloat16), GSIZE * 2 * CH // 16
    )
    groups = []
    for s in range(SPLIT):
        mesh = Mesh(np.asarray(devices[s * GSIZE : (s + 1) * GSIZE]), ("core",))
        sh = NamedSharding(mesh, p_core)
        fn = jax.jit(
            shard_map(
                _body,
                mesh=mesh,
                in_specs=(p_core, p_core, p_core, p_core),
                out_specs=p_core,
                check_rep=False,
            ),
            donate_argnums=(3,),
            keep_unused=True,
        )
        zeros_fn = jax.jit(
            lambda: jnp.zeros((GROWS, N // 8), jnp.uint16),
            out_shardings=sh,
        )
        groups.append(
            dict(
                fn=fn,
                zeros_fn=zeros_fn,
                sh=sh,
                w_dev=jax.device_put(wrow_np, sh),
            )
        )
    _STATE.update(
        groups=groups,
        bf16=ml_dtypes.bfloat16,
        jax=jax,
        qbuf=np.empty((GROWS, N), np.float32),
        q8=np.empty((B, N), np.int8),
    )
    return _STATE


def kernel(output, mean_grad, var_grad, k):
    st = _get_runner()
    jax = st["jax"]

    x = np.ascontiguousarray(np.asarray(output, dtype=np.float32))
    assert x.shape == (B, N), x.shape
    mg = np.asarray(mean_grad, dtype=np.float32)
    vg = np.asarray(var_grad, dtype=np.float32)
    kf = np.float32(k)

    # f32 bounds, bit-matching the reference
    std = np.sqrt(vg, dtype=np.float32)
    ks = (kf * std).astype(np.float32)
    lower = (mg - ks).astype(np.float32)
    upper = (mg + ks).astype(np.float32)

    # quantization scale (min/max: no 128MB temp)
    maxabs = max(float(x.max()), -float(x.min()))
    if maxabs == 0.0:
        maxabs = 1.0
    inv = np.float32(127.0 / maxabs)

    # integer thresholds in step units (margin 2 steps; |g| <= 254)
    los = lower.astype(np.float64) * (127.0 / maxabs)
    ups = upper.astype(np.float64) * (127.0 / maxabs)
    L1 = np.clip(np.ceil(los) + MARGIN, -255, 255)
    U1 = np.clip(np.floor(ups) - MARGIN, -255, 255)
    L0 = np.clip(np.ceil(los) - MARGIN, -255, 255)
    U0 = np.clip(np.floor(ups) + MARGIN, -255, 255)
    thr_np = np.tile(
        np.concatenate([L1, U1, L0, U0]).astype(st["bf16"]), GSIZE
    )

    # pipelined: quantize + upload each group's slab, dispatch all execs
    # (device_put / jit dispatch are async; only np.asarray blocks)
    buf, q8 = st["qbuf"], st["q8"]
    outs = []
    for s, grp in enumerate(st["groups"]):
        r0, r1 = s * GROWS, (s + 1) * GROWS
        np.multiply(x[r0:r1], inv, out=buf)
        np.rint(buf, out=buf)
        np.copyto(q8[r0:r1], buf, casting="unsafe")
        q_dev = jax.device_put(q8[r0:r1], grp["sh"])
        thr_dev = jax.device_put(thr_np, grp["sh"])
        ybuf = grp.pop("ybuf_next", None)
        if ybuf is None:
            ybuf = grp["zeros_fn"]()
        outs.append(grp["fn"](q_dev, thr_dev, grp["w_dev"], ybuf))
        grp["ybuf_next"] = grp["zeros_fn"]()
    for o in outs:
        if hasattr(o, "copy_to_host_async"):
            o.copy_to_host_async()

    y = np.empty_like(x)
    for s, out_dev in enumerate(outs):
        r0, r1 = s * GROWS, (s + 1) * GROWS
        pk = np.asarray(out_dev)  # (GROWS, N//8) uint16
        xh = x[r0:r1]
        # low byte = tight bits, high byte = loose bits (little endian)
        byte_view = pk.view(np.uint8).reshape(GROWS, N // 8, 2)
        t_bytes = np.ascontiguousarray(byte_view[:, :, 0])
        tight = np.unpackbits(t_bytes, axis=1, bitorder="little")
        np.multiply(xh, tight, out=y[r0:r1])

        # exact fixup of the uncertain band (loose & ~tight), sparse decode
        u_bytes = (byte_view[:, :, 1] & ~t_bytes).ravel()
        nzb = np.flatnonzero(u_bytes)
        if nzb.size:
            bits = np.unpackbits(u_bytes[nzb], bitorder="little")
            pos = np.flatnonzero(bits)
            idx = (nzb[pos >> 3] << 3) + (pos & 7)
            xr = xh.ravel()
            cols = idx & (N - 1)
            prev = idx - 1 + ((cols == 0).astype(np.int64) << 13)
            g_ex = xr[idx] - xr[prev]
            keep = (g_ex >= lower[cols]) & (g_ex <= upper[cols])
            y[r0:r1].reshape(-1)[idx] = np.where(
                keep, xr[idx], np.float32(0.0)
            )
    return y


# revision 12
# speedup vs baseline: 9.8970x; 1.1730x over previous
"""Trainium2 Bass kernel for nn_Correction_Module_dense.

Reference computation:
    grad  = x - roll(x, 1, axis=1)            # circular diff along neuron axis
    lower = mean_grad - k*sqrt(var_grad)      # per-neuron
    upper = mean_grad + k*sqrt(var_grad)
    y     = x * (lower <= grad) * (grad <= upper)

End-to-end wall time is dominated by the ~40 MB/s axon tunnel, so the
kernel is built around minimizing bytes on the wire:

  host:   q = rint(x * 127/max|x|)  (int8, 32 MB instead of 128 MB f32)
  device: g = q[i] - q[i-1]  (integer steps, exact in bf16)
          tight = L1 <= g <= U1   (margin +2 steps inside the band)
          loose = L0 <= g <= U0   (margin -2 steps outside the band)
          out   = packed uint16 per 8 neurons: low byte = tight bits,
                  high byte = loose bits  (8 MB total)
  host:   y = where(tight, x, 0); elements with loose & ~tight are within
          +-2 quantization steps of a boundary -> recompute exactly in f32.

Since |g_true/step - g_q| <= 1 + eps, tight => truly in-range and
!loose => truly out-of-range, so after the exact fixup of the uncertain
band the result is bit-identical to the f32 reference.

Sharding: pure data parallel over batch; 8 cores x [512, 8192].
Layout: batch rows -> partitions, neurons -> free axis.  Threshold
vectors are broadcast to 128 partitions once via log2-doubling DMAs.

The jitted shard_map executable is cached across calls (rebuilding it
per call costs a full retrace + PJRT compile); donated output buffers
are created on-device (never shipped over the tunnel).
"""

import numpy as np

import concourse.bass as bass
import concourse.mybir as mybir

B, N = 4096, 8192
N_CORES = 8
ROWS = B // N_CORES   # rows per core
P = 128
NT = ROWS // P        # row tiles per core
CH = 2048             # neuron chunk
NCH = N // CH
NPK = N // 8          # packed uint16s per row
# Uncertain band half-width in quantization steps.  Quantized-diff error
# is <= 1 step + ~6e-5 float slop, so anything > 1.0001 is safe; 1.001
# keeps the definite decisions provably exact while minimizing the
# band population the host must recompute.
MARGIN = 1.001


def build_nc(rows=ROWS, n=N, chunk=CH):
    nt = rows // P
    nch = n // chunk
    npk_t = chunk // 8          # packed u16 per chunk
    grp = chunk // 8            # groups of 8 per chunk
    bf16 = mybir.dt.bfloat16
    f32 = mybir.dt.float32
    i8 = mybir.dt.int8
    u16 = mybir.dt.uint16
    sub = mybir.AluOpType.subtract
    mul = mybir.AluOpType.mult
    is_ge = mybir.AluOpType.is_ge
    is_le = mybir.AluOpType.is_le

    nc = bass.Bass()
    xq = nc.dram_tensor("xq", [rows, n], i8, kind="ExternalInput")
    # thr = [L1 | U1 | L0 | U0], each [n], integer-valued, |.| <= 255
    thr = nc.dram_tensor("thr", [4 * n], bf16, kind="ExternalInput")
    # wrow = 2^j pattern repeating every 16: [1,2,...,32768]*...
    wrow = nc.dram_tensor("wrow", [2 * chunk], bf16, kind="ExternalInput")
    out = nc.dram_tensor("out", [rows, n // 8], u16, kind="ExternalOutput")

    from contextlib import ExitStack

    with ExitStack() as ctx:
        THR = ctx.enter_context(nc.sbuf_tensor("THR", [P, 4 * n], bf16))
        WB = ctx.enter_context(nc.sbuf_tensor("WB", [P, 2 * chunk], bf16))
        XQ = [
            ctx.enter_context(nc.sbuf_tensor(f"XQ{t}", [P, n], i8))
            for t in range(nt)
        ]
        G = [
            ctx.enter_context(nc.sbuf_tensor(f"G{i}", [P, chunk], bf16))
            for i in range(2)
        ]
        A = ctx.enter_context(nc.sbuf_tensor("A", [P, chunk], bf16))
        Bb = ctx.enter_context(nc.sbuf_tensor("Bb", [P, chunk], bf16))
        A2 = ctx.enter_context(nc.sbuf_tensor("A2", [P, chunk], bf16))
        B2 = ctx.enter_context(nc.sbuf_tensor("B2", [P, chunk], bf16))
        TU = [
            ctx.enter_context(nc.sbuf_tensor(f"TU{i}", [P, 2 * chunk], bf16))
            for i in range(2)
        ]
        WM = [
            ctx.enter_context(nc.sbuf_tensor(f"WM{i}", [P, 2 * chunk], bf16))
            for i in range(2)
        ]
        PK = [
            ctx.enter_context(nc.sbuf_tensor(f"PK{i}", [P, npk_t], f32))
            for i in range(2)
        ]
        OUT = [
            ctx.enter_context(nc.sbuf_tensor(f"OUT{i}", [P, n // 8], u16))
            for i in range(2)
        ]

        LB = ctx.enter_context(nc.semaphore("LB"))   # broadcast chain
        LX = [ctx.enter_context(nc.semaphore(f"LX{t}")) for t in range(nt)]
        PS = ctx.enter_context(nc.semaphore("PS"))   # gpsimd chunk progress
        V = ctx.enter_context(nc.semaphore("V"))     # vector chunk progress
        C1 = ctx.enter_context(nc.semaphore("C1"))   # scalar copy progress
        SB = [ctx.enter_context(nc.semaphore(f"SB{i}")) for i in range(2)]
        block = ctx.enter_context(nc.Block())

        n_bcast = 2 * 8  # (1 load + 7 doublings) x 2 tensors
        l_bcast = 16 * n_bcast

        @block.sync
        def _(sync):
            lv = 0
            for vec, t in ((thr, THR), (wrow, WB)):
                sync.dma_start(out=t[0:1, :], in_=vec[None, :]).then_inc(LB, 16)
                lv += 16
                pcnt = 1
                while pcnt < P:
                    sync.wait_ge(LB, lv)
                    sync.dma_start(
                        out=t[pcnt : 2 * pcnt, :], in_=t[0:pcnt, :]
                    ).then_inc(LB, 16)
                    lv += 16
                    pcnt *= 2
            # all x tile loads issued upfront (XQ is nt-buffered)
            for t in range(nt):
                sync.dma_start(
                    out=XQ[t][:], in_=xq[t * P : (t + 1) * P, :]
                ).then_inc(LX[t], 16)
            # stores, one per row tile
            for t in range(nt):
                sync.wait_ge(C1, 4 * t + 4)
                sync.dma_start(
                    out=out[t * P : (t + 1) * P, :], in_=OUT[t % 2][:]
                ).then_inc(SB[t % 2], 16)

        @block.gpsimd
        def _(gpsimd):
            for t in range(nt):
                gpsimd.wait_ge(LX[t], 16)
                xb = XQ[t]
                for c in range(nch):
                    idx = t * nch + c
                    if idx >= 2:
                        gpsimd.wait_ge(V, idx - 1)
                    gb = G[idx % 2]
                    c0 = c * chunk
                    if c == 0:
                        gpsimd.tensor_tensor(
                            gb[:, 1:chunk], xb[:, 1:chunk], xb[:, 0 : chunk - 1], sub
                        )
                        gpsimd.tensor_tensor(
                            gb[:, 0:1], xb[:, 0:1], xb[:, n - 1 : n], sub
                        ).then_inc(PS, 1)
                    else:
                        gpsimd.tensor_tensor(
                            gb[:], xb[:, c0 : c0 + chunk],
                            xb[:, c0 - 1 : c0 + chunk - 1], sub
                        ).then_inc(PS, 1)

        @block.vector
        def _(vector):
            vector.wait_ge(LB, l_bcast)
            for t in range(nt):
                for c in range(nch):
                    idx = t * nch + c
                    c0 = c * chunk
                    gb = G[idx % 2]
                    tu = TU[idx % 2]
                    wm = WM[idx % 2]
                    pk = PK[idx % 2]
                    vector.wait_ge(PS, idx + 1)
                    if idx >= 2:
                        vector.wait_ge(C1, idx - 1)
                    vector.tensor_tensor(
                        A[:], gb[:], THR[:, 0 * n + c0 : 0 * n + c0 + chunk], is_ge
                    )
                    vector.tensor_tensor(
                        Bb[:], gb[:], THR[:, 1 * n + c0 : 1 * n + c0 + chunk], is_le
                    )
                    vector.tensor_tensor(
                        A2[:], gb[:], THR[:, 2 * n + c0 : 2 * n + c0 + chunk], is_ge
                    )
                    vector.tensor_tensor(
                        B2[:], gb[:], THR[:, 3 * n + c0 : 3 * n + c0 + chunk], is_le
                    )
                    vector.drain()
                    tu3 = tu[:].rearrange("p (g k) -> p g k", k=16)
                    vector.tensor_tensor(
                        tu3[:, :, 0:8],
                        A[:].rearrange("p (g k) -> p g k", k=8),
                        Bb[:].rearrange("p (g k) -> p g k", k=8),
                        mul,
                    )
                    vector.tensor_tensor(
                        tu3[:, :, 8:16],
                        A2[:].rearrange("p (g k) -> p g k", k=8),
                        B2[:].rearrange("p (g k) -> p g k", k=8),
                        mul,
                    )
                    vector.drain()
                    vector.tensor_tensor(wm[:], tu[:], WB[:], mul)
                    vector.drain()
                    vector.tensor_reduce(
                        pk[:],
                        wm[:].rearrange("p (g k) -> p g k", k=16),
                        mybir.AxisListType.X,
                        mybir.AluOpType.add,
                    ).then_inc(V, 1)

        @block.scalar
        def _(scalar):
            for t in range(nt):
                for c in range(nch):
                    idx = t * nch + c
                    scalar.wait_ge(V, idx + 1)
                    if c == 0 and t >= 2:
                        scalar.wait_ge(SB[t % 2], 16 * (t // 2))
                    scalar.copy(
                        OUT[t % 2][:, c * npk_t : (c + 1) * npk_t],
                        PK[idx % 2][:],
                    ).then_inc(C1, 1)

    return nc


_STATE = {}
SPLIT = 2                      # pipeline stages (device groups)
GSIZE = N_CORES // SPLIT       # cores per group
GROWS = B // SPLIT             # batch rows per group


def _get_runner():
    """Build (once) the cached jitted shard_map executables, one per
    device group (the batch is pipelined across groups so host work
    overlaps the ~40 MB/s tunnel transfers)."""
    if "groups" in _STATE:
        return _STATE

    import jax
    import jax.numpy as jnp
    from jax.sharding import Mesh, PartitionSpec, NamedSharding
    from concourse import bass2jax

    try:
        from jax.experimental.shard_map import shard_map
    except ImportError:
        from jax.sharding import shard_map

    bass2jax.install_neuronx_cc_hook()

    nc = build_nc()
    assert nc.dbg_addr is None
    pid_name = nc.partition_id_tensor.name if nc.partition_id_tensor else None

    in_names = []
    out_names = []
    out_avals = []
    for alloc in nc.m.functions[0].allocations:
        if not isinstance(alloc, mybir.MemoryLocationSet):
            continue
        name = alloc.memorylocations[0].name
        if alloc.kind == "ExternalInput":
            if name != pid_name:
                in_names.append(name)
        elif alloc.kind == "ExternalOutput":
            out_names.append(name)
            out_avals.append(
                jax.core.ShapedArray(
                    tuple(alloc.tensor_shape), mybir.dt.np(alloc.dtype)
                )
            )
    assert in_names == ["xq", "thr", "wrow"], in_names
    assert out_names == ["out"], out_names
    all_in_names = tuple(in_names) + tuple(out_names)
    if pid_name is not None:
        all_in_names = all_in_names + (pid_name,)

    def _body(xq_a, thr_a, wrow_a, ybuf_a):
        operands = [xq_a, thr_a, wrow_a, ybuf_a]
        if pid_name is not None:
            operands.append(bass2jax.partition_id_tensor())
        outs = bass2jax._bass_exec_p.bind(
            *operands,
            out_avals=tuple(out_avals),
            in_names=all_in_names,
            out_names=tuple(out_names),
            lowering_input_output_aliases=(),
            sim_require_finite=True,
            sim_require_nnan=True,
            nc=nc,
        )
        return outs[0]

    devices = jax.devices()[:N_CORES]
    assert len(devices) == N_CORES
    p_core = PartitionSpec("core")

    import ml_dtypes

    wrow_np = np.tile(
        (2.0 ** np.arange(16)).astype(ml_dtypes.bfloat16), GSIZE * 2 * CH // 16
    )
    groups = []
    for s in range(SPLIT):
        mesh = Mesh(np.asarray(devices[s * GSIZE : (s + 1) * GSIZE]), ("core",))
        sh = NamedSharding(mesh, p_core)
        fn = jax.jit(
            shard_map(
                _body,
                mesh=mesh,
                in_specs=(p_core, p_core, p_core, p_core),
                out_specs=p_core,
                check_rep=False,
            ),
            donate_argnums=(3,),
            keep_unused=True,
        )
        zeros_fn = jax.jit(
            lambda: jnp.zeros((GROWS, N // 8), jnp.uint16),
            out_shardings=sh,
        )
        groups.append(
            dict(
                fn=fn,
                zeros_fn=zeros_fn,
                sh=sh,
                w_dev=jax.device_put(wrow_np, sh),
            )
        )
    _STATE.update(
        groups=groups,
        bf16=ml_dtypes.bfloat16,
        jax=jax,
        qbuf=np.empty((GROWS, N), np.float32),
        q8=np.empty((B, N), np.int8),
    )
    return _STATE


def _warmup():
    """Compile the jitted executables and run one dummy exec per group so
    the first real kernel() call pays no compile cost.  Best-effort."""
    try:
        st = _get_runner()
        thr0 = np.zeros(GSIZE * 4 * N, st["bf16"])
        q0 = np.zeros((GROWS, N), np.int8)
        outs = []
        for grp in st["groups"]:
            q_dev = st["jax"].device_put(q0, grp["sh"])
            thr_dev = st["jax"].device_put(thr0, grp["sh"])
            outs.append(grp["fn"](q_dev, thr_dev, grp["w_dev"], grp["zeros_fn"]()))
            grp["ybuf_next"] = grp["zeros_fn"]()
        for o in outs:
            np.asarray(o)
    except Exception:
        _STATE.clear()


_warmup()


def kernel(output, mean_grad, var_grad, k):
    import os
    import time as _time

    _tt = [] if os.environ.get("KBENCH") else None

    def _mark(label):
        if _tt is not None:
            _tt.append((label, _time.time()))

    st = _get_runner()
    jax = st["jax"]
    _mark("start")

    x = np.ascontiguousarray(np.asarray(output, dtype=np.float32))
    assert x.shape == (B, N), x.shape
    mg = np.asarray(mean_grad, dtype=np.float32)
    vg = np.asarray(var_grad, dtype=np.float32)
    kf = np.float32(k)

    # f32 bounds, bit-matching the reference
    std = np.sqrt(vg, dtype=np.float32)
    ks = (kf * std).astype(np.float32)
    lower = (mg - ks).astype(np.float32)
    upper = (mg + ks).astype(np.float32)

    # quantization scale (min/max: no 128MB temp)
    maxabs = max(float(x.max()), -float(x.min()))
    _mark("maxabs")
    if maxabs == 0.0:
        maxabs = 1.0
    inv = np.float32(127.0 / maxabs)

    # integer thresholds in step units (see MARGIN; |g| <= 254)
    los = lower.astype(np.float64) * (127.0 / maxabs)
    ups = upper.astype(np.float64) * (127.0 / maxabs)
    L1 = np.clip(np.ceil(los) + MARGIN, -255, 255)
    U1 = np.clip(np.floor(ups) - MARGIN, -255, 255)
    L0 = np.clip(np.ceil(los) - MARGIN, -255, 255)
    U0 = np.clip(np.floor(ups) + MARGIN, -255, 255)
    thr_np = np.tile(
        np.concatenate([L1, U1, L0, U0]).astype(st["bf16"]), GSIZE
    )

    # pipelined: quantize + upload each group's slab, dispatch all execs
    # (device_put / jit dispatch are async; only np.asarray blocks)
    buf, q8 = st["qbuf"], st["q8"]
    outs = []
    for s, grp in enumerate(st["groups"]):
        r0, r1 = s * GROWS, (s + 1) * GROWS
        np.multiply(x[r0:r1], inv, out=buf)
        np.rint(buf, out=buf)
        np.copyto(q8[r0:r1], buf, casting="unsafe")
        q_dev = jax.device_put(q8[r0:r1], grp["sh"])
        thr_dev = jax.device_put(thr_np, grp["sh"])
        ybuf = grp.pop("ybuf_next", None)
        if ybuf is None:
            ybuf = grp["zeros_fn"]()
        outs.append(grp["fn"](q_dev, thr_dev, grp["w_dev"], ybuf))
        _mark(f"issued{s}")
    for o in outs:
        if hasattr(o, "copy_to_host_async"):
            o.copy_to_host_async()
    # pre-create next call's donated output buffers (off the issue path)
    for grp in st["groups"]:
        if "ybuf_next" not in grp:
            grp["ybuf_next"] = grp["zeros_fn"]()

    def _decode(pk, xh, yh):
        # low byte = tight bits, high byte = loose bits (little endian)
        rows = pk.shape[0]
        byte_view = pk.view(np.uint8).reshape(rows, N // 8, 2)
        t_bytes = np.ascontiguousarray(byte_view[:, :, 0])
        tight = np.unpackbits(t_bytes, axis=1, bitorder="little")
        np.multiply(xh, tight, out=yh)

        # exact fixup of the uncertain band (loose & ~tight), sparse decode
        u_bytes = (byte_view[:, :, 1] & ~t_bytes).ravel()
        nzb = np.flatnonzero(u_bytes)
        if nzb.size:
            bits = np.unpackbits(u_bytes[nzb], bitorder="little")
            pos = np.flatnonzero(bits)
            idx = (nzb[pos >> 3] << 3) + (pos & 7)
            xr = xh.ravel()
            cols = idx & (N - 1)
            prev = idx - 1 + ((cols == 0).astype(np.int64) << 13)
            g_ex = xr[idx] - xr[prev]
            keep = (g_ex >= lower[cols]) & (g_ex <= upper[cols])
            yh.reshape(-1)[idx] = np.where(keep, xr[idx], np.float32(0.0))

    y = np.empty_like(x)
    for s, out_dev in enumerate(outs):
        g0 = s * GROWS
        _mark(f"prefetch{s}")
        shards = getattr(out_dev, "addressable_shards", None)
        if shards is not None and len(shards) > 1:
            for shd in sorted(shards, key=lambda q: q.index[0].start or 0):
                r0 = g0 + (shd.index[0].start or 0)
                pk = np.asarray(shd.data)
                _decode(pk, x[r0 : r0 + pk.shape[0]], y[r0 : r0 + pk.shape[0]])
        else:
            pk = np.asarray(out_dev)
            _decode(pk, x[g0 : g0 + GROWS], y[g0 : g0 + GROWS])
        _mark(f"post{s}")
    if _tt is not None:
        t0 = _tt[0][1]
        _STATE["last_times"] = [(l, t - t0) for l, t in _tt]
    return y


# revision 13
# speedup vs baseline: 10.0812x; 1.0186x over previous
"""Trainium2 Bass kernel for nn_Correction_Module_dense.

Reference computation:
    grad  = x - roll(x, 1, axis=1)            # circular diff along neuron axis
    lower = mean_grad - k*sqrt(var_grad)      # per-neuron
    upper = mean_grad + k*sqrt(var_grad)
    y     = x * (lower <= grad) * (grad <= upper)

End-to-end wall time is dominated by the ~40 MB/s axon tunnel, so the
kernel is built around minimizing bytes on the wire:

  host:   q = rint(x * 127/max|x|)  (int8, 32 MB instead of 128 MB f32)
  device: g = q[i] - q[i-1]  (integer steps, exact in bf16)
          tight = L1 <= g <= U1   (margin +2 steps inside the band)
          loose = L0 <= g <= U0   (margin -2 steps outside the band)
          out   = packed uint16 per 8 neurons: low byte = tight bits,
                  high byte = loose bits  (8 MB total)
  host:   y = where(tight, x, 0); elements with loose & ~tight are within
          +-2 quantization steps of a boundary -> recompute exactly in f32.

Since |g_true/step - g_q| <= 1 + eps, tight => truly in-range and
!loose => truly out-of-range, so after the exact fixup of the uncertain
band the result is bit-identical to the f32 reference.

Sharding: pure data parallel over batch; 8 cores x [512, 8192].
Layout: batch rows -> partitions, neurons -> free axis.  Threshold
vectors are broadcast to 128 partitions once via log2-doubling DMAs.

The jitted shard_map executable is cached across calls (rebuilding it
per call costs a full retrace + PJRT compile); donated output buffers
are created on-device (never shipped over the tunnel).
"""

import numpy as np

import concourse.bass as bass
import concourse.mybir as mybir

B, N = 4096, 8192
N_CORES = 8
ROWS = B // N_CORES   # rows per core
P = 128
NT = ROWS // P        # row tiles per core
CH = 2048             # neuron chunk
NCH = N // CH
NPK = N // 8          # packed uint16s per row
# Uncertain band half-width in quantization steps.  Quantized-diff error
# is <= 1 step + ~6e-5 float slop, so anything > 1.0001 is safe; 1.001
# keeps the definite decisions provably exact while minimizing the
# band population the host must recompute.
MARGIN = 1.001


def build_nc(rows=ROWS, n=N, chunk=CH):
    nt = rows // P
    nch = n // chunk
    npk_t = chunk // 8          # packed u16 per chunk
    grp = chunk // 8            # groups of 8 per chunk
    bf16 = mybir.dt.bfloat16
    f32 = mybir.dt.float32
    i8 = mybir.dt.int8
    u16 = mybir.dt.uint16
    sub = mybir.AluOpType.subtract
    mul = mybir.AluOpType.mult
    is_ge = mybir.AluOpType.is_ge
    is_le = mybir.AluOpType.is_le

    nc = bass.Bass()
    xq = nc.dram_tensor("xq", [rows, n], i8, kind="ExternalInput")
    # thr = [L1 | U1 | L0 | U0], each [n], integer-valued, |.| <= 255
    thr = nc.dram_tensor("thr", [4 * n], bf16, kind="ExternalInput")
    # wrow = 2^j pattern repeating every 16: [1,2,...,32768]*...
    wrow = nc.dram_tensor("wrow", [2 * chunk], bf16, kind="ExternalInput")
    out = nc.dram_tensor("out", [rows, n // 8], u16, kind="ExternalOutput")

    from contextlib import ExitStack

    with ExitStack() as ctx:
        THR = ctx.enter_context(nc.sbuf_tensor("THR", [P, 4 * n], bf16))
        WB = ctx.enter_context(nc.sbuf_tensor("WB", [P, 2 * chunk], bf16))
        XQ = [
            ctx.enter_context(nc.sbuf_tensor(f"XQ{t}", [P, n], i8))
            for t in range(nt)
        ]
        G = [
            ctx.enter_context(nc.sbuf_tensor(f"G{i}", [P, chunk], bf16))
            for i in range(2)
        ]
        A = ctx.enter_context(nc.sbuf_tensor("A", [P, chunk], bf16))
        Bb = ctx.enter_context(nc.sbuf_tensor("Bb", [P, chunk], bf16))
        A2 = ctx.enter_context(nc.sbuf_tensor("A2", [P, chunk], bf16))
        B2 = ctx.enter_context(nc.sbuf_tensor("B2", [P, chunk], bf16))
        TU = [
            ctx.enter_context(nc.sbuf_tensor(f"TU{i}", [P, 2 * chunk], bf16))
            for i in range(2)
        ]
        WM = [
            ctx.enter_context(nc.sbuf_tensor(f"WM{i}", [P, 2 * chunk], bf16))
            for i in range(2)
        ]
        PK = [
            ctx.enter_context(nc.sbuf_tensor(f"PK{i}", [P, npk_t], f32))
            for i in range(2)
        ]
        OUT = [
            ctx.enter_context(nc.sbuf_tensor(f"OUT{i}", [P, n // 8], u16))
            for i in range(2)
        ]

        LB = ctx.enter_context(nc.semaphore("LB"))   # broadcast chain
        LX = [ctx.enter_context(nc.semaphore(f"LX{t}")) for t in range(nt)]
        PS = ctx.enter_context(nc.semaphore("PS"))   # gpsimd chunk progress
        V = ctx.enter_context(nc.semaphore("V"))     # vector chunk progress
        C1 = ctx.enter_context(nc.semaphore("C1"))   # scalar copy progress
        SB = [ctx.enter_context(nc.semaphore(f"SB{i}")) for i in range(2)]
        block = ctx.enter_context(nc.Block())

        n_bcast = 2 * 8  # (1 load + 7 doublings) x 2 tensors
        l_bcast = 16 * n_bcast

        @block.sync
        def _(sync):
            lv = 0
            for vec, t in ((thr, THR), (wrow, WB)):
                sync.dma_start(out=t[0:1, :], in_=vec[None, :]).then_inc(LB, 16)
                lv += 16
                pcnt = 1
                while pcnt < P:
                    sync.wait_ge(LB, lv)
                    sync.dma_start(
                        out=t[pcnt : 2 * pcnt, :], in_=t[0:pcnt, :]
                    ).then_inc(LB, 16)
                    lv += 16
                    pcnt *= 2
            # all x tile loads issued upfront (XQ is nt-buffered)
            for t in range(nt):
                sync.dma_start(
                    out=XQ[t][:], in_=xq[t * P : (t + 1) * P, :]
                ).then_inc(LX[t], 16)
            # stores, one per row tile
            for t in range(nt):
                sync.wait_ge(C1, 4 * t + 4)
                sync.dma_start(
                    out=out[t * P : (t + 1) * P, :], in_=OUT[t % 2][:]
                ).then_inc(SB[t % 2], 16)

        @block.gpsimd
        def _(gpsimd):
            for t in range(nt):
                gpsimd.wait_ge(LX[t], 16)
                xb = XQ[t]
                for c in range(nch):
                    idx = t * nch + c
                    if idx >= 2:
                        gpsimd.wait_ge(V, idx - 1)
                    gb = G[idx % 2]
                    c0 = c * chunk
                    if c == 0:
                        gpsimd.tensor_tensor(
                            gb[:, 1:chunk], xb[:, 1:chunk], xb[:, 0 : chunk - 1], sub
                        )
                        gpsimd.tensor_tensor(
                            gb[:, 0:1], xb[:, 0:1], xb[:, n - 1 : n], sub
                        ).then_inc(PS, 1)
                    else:
                        gpsimd.tensor_tensor(
                            gb[:], xb[:, c0 : c0 + chunk],
                            xb[:, c0 - 1 : c0 + chunk - 1], sub
                        ).then_inc(PS, 1)

        @block.vector
        def _(vector):
            vector.wait_ge(LB, l_bcast)
            for t in range(nt):
                for c in range(nch):
                    idx = t * nch + c
                    c0 = c * chunk
                    gb = G[idx % 2]
                    tu = TU[idx % 2]
                    wm = WM[idx % 2]
                    pk = PK[idx % 2]
                    vector.wait_ge(PS, idx + 1)
                    if idx >= 2:
                        vector.wait_ge(C1, idx - 1)
                    vector.tensor_tensor(
                        A[:], gb[:], THR[:, 0 * n + c0 : 0 * n + c0 + chunk], is_ge
                    )
                    vector.tensor_tensor(
                        Bb[:], gb[:], THR[:, 1 * n + c0 : 1 * n + c0 + chunk], is_le
                    )
                    vector.tensor_tensor(
                        A2[:], gb[:], THR[:, 2 * n + c0 : 2 * n + c0 + chunk], is_ge
                    )
                    vector.tensor_tensor(
                        B2[:], gb[:], THR[:, 3 * n + c0 : 3 * n + c0 + chunk], is_le
                    )
                    vector.drain()
                    tu3 = tu[:].rearrange("p (g k) -> p g k", k=16)
                    vector.tensor_tensor(
                        tu3[:, :, 0:8],
                        A[:].rearrange("p (g k) -> p g k", k=8),
                        Bb[:].rearrange("p (g k) -> p g k", k=8),
                        mul,
                    )
                    vector.tensor_tensor(
                        tu3[:, :, 8:16],
                        A2[:].rearrange("p (g k) -> p g k", k=8),
                        B2[:].rearrange("p (g k) -> p g k", k=8),
                        mul,
                    )
                    vector.drain()
                    vector.tensor_tensor(wm[:], tu[:], WB[:], mul)
                    vector.drain()
                    vector.tensor_reduce(
                        pk[:],
                        wm[:].rearrange("p (g k) -> p g k", k=16),
                        mybir.AxisListType.X,
                        mybir.AluOpType.add,
                    ).then_inc(V, 1)

        @block.scalar
        def _(scalar):
            for t in range(nt):
                for c in range(nch):
                    idx = t * nch + c
                    scalar.wait_ge(V, idx + 1)
                    if c == 0 and t >= 2:
                        scalar.wait_ge(SB[t % 2], 16 * (t // 2))
                    scalar.copy(
                        OUT[t % 2][:, c * npk_t : (c + 1) * npk_t],
                        PK[idx % 2][:],
                    ).then_inc(C1, 1)

    return nc


_STATE = {}
SPLIT = 2                      # pipeline stages (device groups)
GSIZE = N_CORES // SPLIT       # cores per group
GROWS = B // SPLIT             # batch rows per group


def _get_runner():
    """Build (once) the cached jitted shard_map executables, one per
    device group (the batch is pipelined across groups so host work
    overlaps the ~40 MB/s tunnel transfers)."""
    if "groups" in _STATE:
        return _STATE

    import jax
    import jax.numpy as jnp
    from jax.sharding import Mesh, PartitionSpec, NamedSharding
    from concourse import bass2jax

    try:
        from jax.experimental.shard_map import shard_map
    except ImportError:
        from jax.sharding import shard_map

    bass2jax.install_neuronx_cc_hook()

    nc = build_nc()
    assert nc.dbg_addr is None
    pid_name = nc.partition_id_tensor.name if nc.partition_id_tensor else None

    in_names = []
    out_names = []
    out_avals = []
    for alloc in nc.m.functions[0].allocations:
        if not isinstance(alloc, mybir.MemoryLocationSet):
            continue
        name = alloc.memorylocations[0].name
        if alloc.kind == "ExternalInput":
            if name != pid_name:
                in_names.append(name)
        elif alloc.kind == "ExternalOutput":
            out_names.append(name)
            out_avals.append(
                jax.core.ShapedArray(
                    tuple(alloc.tensor_shape), mybir.dt.np(alloc.dtype)
                )
            )
    assert in_names == ["xq", "thr", "wrow"], in_names
    assert out_names == ["out"], out_names
    all_in_names = tuple(in_names) + tuple(out_names)
    if pid_name is not None:
        all_in_names = all_in_names + (pid_name,)

    def _body(xq_a, thr_a, wrow_a, ybuf_a):
        operands = [xq_a, thr_a, wrow_a, ybuf_a]
        if pid_name is not None:
            operands.append(bass2jax.partition_id_tensor())
        outs = bass2jax._bass_exec_p.bind(
            *operands,
            out_avals=tuple(out_avals),
            in_names=all_in_names,
            out_names=tuple(out_names),
            lowering_input_output_aliases=(),
            sim_require_finite=True,
            sim_require_nnan=True,
            nc=nc,
        )
        return outs[0]

    devices = jax.devices()[:N_CORES]
    assert len(devices) == N_CORES
    p_core = PartitionSpec("core")

    import ml_dtypes

    wrow_np = np.tile(
        (2.0 ** np.arange(16)).astype(ml_dtypes.bfloat16), GSIZE * 2 * CH // 16
    )
    groups = []
    for s in range(SPLIT):
        mesh = Mesh(np.asarray(devices[s * GSIZE : (s + 1) * GSIZE]), ("core",))
        sh = NamedSharding(mesh, p_core)
        fn = jax.jit(
            shard_map(
                _body,
                mesh=mesh,
                in_specs=(p_core, p_core, p_core, p_core),
                out_specs=p_core,
                check_rep=False,
            ),
            donate_argnums=(3,),
            keep_unused=True,
        )
        zeros_fn = jax.jit(
            lambda: jnp.zeros((GROWS, N // 8), jnp.uint16),
            out_shardings=sh,
        )
        groups.append(
            dict(
                fn=fn,
                zeros_fn=zeros_fn,
                sh=sh,
                w_dev=jax.device_put(wrow_np, sh),
            )
        )
    _STATE.update(
        groups=groups,
        bf16=ml_dtypes.bfloat16,
        jax=jax,
        qbuf=np.empty((GROWS, N), np.float32),
        q8=np.empty((B, N), np.int8),
    )
    return _STATE


def _warmup():
    """Compile the jitted executables and run one dummy exec per group so
    the first real kernel() call pays no compile cost.  Best-effort."""
    try:
        st = _get_runner()
        thr0 = np.zeros(GSIZE * 4 * N, st["bf16"])
        q0 = np.zeros((GROWS, N), np.int8)
        outs = []
        for grp in st["groups"]:
            q_dev = st["jax"].device_put(q0, grp["sh"])
            thr_dev = st["jax"].device_put(thr0, grp["sh"])
            outs.append(grp["fn"](q_dev, thr_dev, grp["w_dev"], grp["zeros_fn"]()))
            grp["ybuf_next"] = grp["zeros_fn"]()
        for o in outs:
            np.asarray(o)
    except Exception:
        _STATE.clear()


_warmup()


def kernel(output, mean_grad, var_grad, k):
    import os
    import time as _time

    _tt = [] if os.environ.get("KBENCH") else None

    def _mark(label):
        if _tt is not None:
            _tt.append((label, _time.time()))

    st = _get_runner()
    jax = st["jax"]
    _mark("start")

    x = np.ascontiguousarray(np.asarray(output, dtype=np.float32))
    assert x.shape == (B, N), x.shape
    mg = np.asarray(mean_grad, dtype=np.float32)
    vg = np.asarray(var_grad, dtype=np.float32)
    kf = np.float32(k)

    # f32 bounds, bit-matching the reference
    std = np.sqrt(vg, dtype=np.float32)
    ks = (kf * std).astype(np.float32)
    lower = (mg - ks).astype(np.float32)
    upper = (mg + ks).astype(np.float32)

    # quantization scale (min/max: no 128MB temp)
    maxabs = max(float(x.max()), -float(x.min()))
    _mark("maxabs")
    if maxabs == 0.0:
        maxabs = 1.0
    inv = np.float32(127.0 / maxabs)

    # integer thresholds in step units (see MARGIN; |g| <= 254)
    los = lower.astype(np.float64) * (127.0 / maxabs)
    ups = upper.astype(np.float64) * (127.0 / maxabs)
    L1 = np.clip(np.ceil(los + MARGIN), -255, 255)
    U1 = np.clip(np.floor(ups - MARGIN), -255, 255)
    L0 = np.clip(np.ceil(los - MARGIN), -255, 255)
    U0 = np.clip(np.floor(ups + MARGIN), -255, 255)
    thr_np = np.tile(
        np.concatenate([L1, U1, L0, U0]).astype(st["bf16"]), GSIZE
    )

    # pipelined: quantize + upload each group's slab, dispatch all execs
    # (device_put / jit dispatch are async; only np.asarray blocks)
    buf, q8 = st["qbuf"], st["q8"]
    outs = []
    for s, grp in enumerate(st["groups"]):
        r0, r1 = s * GROWS, (s + 1) * GROWS
        np.multiply(x[r0:r1], inv, out=buf)
        np.rint(buf, out=buf)
        np.copyto(q8[r0:r1], buf, casting="unsafe")
        q_dev = jax.device_put(q8[r0:r1], grp["sh"])
        thr_dev = jax.device_put(thr_np, grp["sh"])
        ybuf = grp.pop("ybuf_next", None)
        if ybuf is None:
            ybuf = grp["zeros_fn"]()
        outs.append(grp["fn"](q_dev, thr_dev, grp["w_dev"], ybuf))
        _mark(f"issued{s}")
    for o in outs:
        if hasattr(o, "copy_to_host_async"):
            o.copy_to_host_async()
    # pre-create next call's donated output buffers (off the issue path)
    for grp in st["groups"]:
        if "ybuf_next" not in grp:
            grp["ybuf_next"] = grp["zeros_fn"]()

    def _decode(pk, xh, yh):
        # low byte = tight bits, high byte = loose bits (little endian)
        rows = pk.shape[0]
        byte_view = pk.view(np.uint8).reshape(rows, N // 8, 2)
        t_bytes = np.ascontiguousarray(byte_view[:, :, 0])
        tight = np.unpackbits(t_bytes, axis=1, bitorder="little")
        np.multiply(xh, tight, out=yh)

        # exact fixup of the uncertain band (loose & ~tight), sparse decode
        u_bytes = (byte_view[:, :, 1] & ~t_bytes).ravel()
        nzb = np.flatnonzero(u_bytes)
        if nzb.size:
            bits = np.unpackbits(u_bytes[nzb], bitorder="little")
            pos = np.flatnonzero(bits)
            idx = (nzb[pos >> 3] << 3) + (pos & 7)
            xr = xh.ravel()
            cols = idx & (N - 1)
            prev = idx - 1 + ((cols == 0).astype(np.int64) << 13)
            g_ex = xr[idx] - xr[prev]
            keep = (g_ex >= lower[cols]) & (g_ex <= upper[cols])
            yh.reshape(-1)[idx] = np.where(keep, xr[idx], np.float32(0.0))

    y = np.empty_like(x)
    for s, out_dev in enumerate(outs):
        g0 = s * GROWS
        _mark(f"prefetch{s}")
        shards = getattr(out_dev, "addressable_shards", None)
        if shards is not None and len(shards) > 1:
            for shd in sorted(shards, key=lambda q: q.index[0].start or 0):
                r0 = g0 + (shd.index[0].start or 0)
                pk = np.asarray(shd.data)
                _decode(pk, x[r0 : r0 + pk.shape[0]], y[r0 : r0 + pk.shape[0]])
        else:
            pk = np.asarray(out_dev)
            _decode(pk, x[g0 : g0 + GROWS], y[g0 : g0 + GROWS])
        _mark(f"post{s}")
    if _tt is not None:
        t0 = _tt[0][1]
        _STATE["last_times"] = [(l, t - t0) for l, t in _tt]
    return y


# revision 19
# speedup vs baseline: 10.1695x; 1.0088x over previous
"""Trainium2 Bass kernel for nn_Correction_Module_dense.

Reference computation:
    grad  = x - roll(x, 1, axis=1)            # circular diff along neuron axis
    lower = mean_grad - k*sqrt(var_grad)      # per-neuron
    upper = mean_grad + k*sqrt(var_grad)
    y     = x * (lower <= grad) * (grad <= upper)

End-to-end wall time is dominated by the ~40 MB/s axon tunnel, so the
kernel is built around minimizing bytes on the wire:

  host:   q = rint(x * 127/max|x|)  (int8, 32 MB instead of 128 MB f32)
  device: g = q[i] - q[i-1]  (integer steps, exact in bf16)
          tight = L1 <= g <= U1   (margin +2 steps inside the band)
          loose = L0 <= g <= U0   (margin -2 steps outside the band)
          out   = packed uint16 per 8 neurons: low byte = tight bits,
                  high byte = loose bits  (8 MB total)
  host:   y = where(tight, x, 0); elements with loose & ~tight are within
          +-2 quantization steps of a boundary -> recompute exactly in f32.

Since |g_true/step - g_q| <= 1 + eps, tight => truly in-range and
!loose => truly out-of-range, so after the exact fixup of the uncertain
band the result is bit-identical to the f32 reference.

Sharding: pure data parallel over batch; 8 cores x [512, 8192].
Layout: batch rows -> partitions, neurons -> free axis.  Threshold
vectors are broadcast to 128 partitions once via log2-doubling DMAs.

The jitted shard_map executable is cached across calls (rebuilding it
per call costs a full retrace + PJRT compile); donated output buffers
are created on-device (never shipped over the tunnel).
"""

import numpy as np

import concourse.bass as bass
import concourse.mybir as mybir

B, N = 4096, 8192
N_CORES = 8
ROWS = B // N_CORES   # rows per core
P = 128
NT = ROWS // P        # row tiles per core
CH = 2048             # neuron chunk
NCH = N // CH
NPK = N // 8          # packed uint16s per row
# Uncertain band half-width in quantization steps.  Quantized-diff error
# is <= 1 step + ~6e-5 float slop, so anything > 1.0001 is safe; 1.001
# keeps the definite decisions provably exact while minimizing the
# band population the host must recompute.
MARGIN = 1.001


def build_nc(rows=ROWS, n=N, chunk=CH):
    nt = rows // P
    nch = n // chunk
    npk_t = chunk // 8          # packed u16 per chunk
    grp = chunk // 8            # groups of 8 per chunk
    bf16 = mybir.dt.bfloat16
    f32 = mybir.dt.float32
    i8 = mybir.dt.int8
    u16 = mybir.dt.uint16
    sub = mybir.AluOpType.subtract
    mul = mybir.AluOpType.mult
    is_ge = mybir.AluOpType.is_ge
    is_le = mybir.AluOpType.is_le

    nc = bass.Bass()
    xq = nc.dram_tensor("xq", [rows, n], i8, kind="ExternalInput")
    # thr = [L1 | U1 | L0 | U0], each [n], integer-valued, |.| <= 255
    thr = nc.dram_tensor("thr", [4 * n], bf16, kind="ExternalInput")
    # wrow = 2^j pattern repeating every 16: [1,2,...,32768]*...
    wrow = nc.dram_tensor("wrow", [2 * chunk], bf16, kind="ExternalInput")
    out = nc.dram_tensor("out", [rows, n // 8], u16, kind="ExternalOutput")

    from contextlib import ExitStack

    with ExitStack() as ctx:
        THR = ctx.enter_context(nc.sbuf_tensor("THR", [P, 4 * n], bf16))
        WB = ctx.enter_context(nc.sbuf_tensor("WB", [P, 2 * chunk], bf16))
        XQ = [
            ctx.enter_context(nc.sbuf_tensor(f"XQ{t}", [P, n], i8))
            for t in range(nt)
        ]
        G = [
            ctx.enter_context(nc.sbuf_tensor(f"G{i}", [P, chunk], bf16))
            for i in range(2)
        ]
        A = ctx.enter_context(nc.sbuf_tensor("A", [P, chunk], bf16))
        Bb = ctx.enter_context(nc.sbuf_tensor("Bb", [P, chunk], bf16))
        A2 = ctx.enter_context(nc.sbuf_tensor("A2", [P, chunk], bf16))
        B2 = ctx.enter_context(nc.sbuf_tensor("B2", [P, chunk], bf16))
        TU = [
            ctx.enter_context(nc.sbuf_tensor(f"TU{i}", [P, 2 * chunk], bf16))
            for i in range(2)
        ]
        WM = [
            ctx.enter_context(nc.sbuf_tensor(f"WM{i}", [P, 2 * chunk], bf16))
            for i in range(2)
        ]
        PK = [
            ctx.enter_context(nc.sbuf_tensor(f"PK{i}", [P, npk_t], f32))
            for i in range(2)
        ]
        OUT = [
            ctx.enter_context(nc.sbuf_tensor(f"OUT{i}", [P, n // 8], u16))
            for i in range(2)
        ]

        LB = ctx.enter_context(nc.semaphore("LB"))   # broadcast chain
        LX = [ctx.enter_context(nc.semaphore(f"LX{t}")) for t in range(nt)]
        PS = ctx.enter_context(nc.semaphore("PS"))   # gpsimd chunk progress
        V = ctx.enter_context(nc.semaphore("V"))     # vector chunk progress
        C1 = ctx.enter_context(nc.semaphore("C1"))   # scalar copy progress
        SB = [ctx.enter_context(nc.semaphore(f"SB{i}")) for i in range(2)]
        block = ctx.enter_context(nc.Block())

        n_bcast = 2 * 8  # (1 load + 7 doublings) x 2 tensors
        l_bcast = 16 * n_bcast

        @block.sync
        def _(sync):
            lv = 0
            for vec, t in ((thr, THR), (wrow, WB)):
                sync.dma_start(out=t[0:1, :], in_=vec[None, :]).then_inc(LB, 16)
                lv += 16
                pcnt = 1
                while pcnt < P:
                    sync.wait_ge(LB, lv)
                    sync.dma_start(
                        out=t[pcnt : 2 * pcnt, :], in_=t[0:pcnt, :]
                    ).then_inc(LB, 16)
                    lv += 16
                    pcnt *= 2
            # all x tile loads issued upfront (XQ is nt-buffered)
            for t in range(nt):
                sync.dma_start(
                    out=XQ[t][:], in_=xq[t * P : (t + 1) * P, :]
                ).then_inc(LX[t], 16)
            # stores, one per row tile
            for t in range(nt):
                sync.wait_ge(C1, 4 * t + 4)
                sync.dma_start(
                    out=out[t * P : (t + 1) * P, :], in_=OUT[t % 2][:]
                ).then_inc(SB[t % 2], 16)

        @block.gpsimd
        def _(gpsimd):
            for t in range(nt):
                gpsimd.wait_ge(LX[t], 16)
                xb = XQ[t]
                for c in range(nch):
                    idx = t * nch + c
                    if idx >= 2:
                        gpsimd.wait_ge(V, idx - 1)
                    gb = G[idx % 2]
                    c0 = c * chunk
                    if c == 0:
                        gpsimd.tensor_tensor(
                            gb[:, 1:chunk], xb[:, 1:chunk], xb[:, 0 : chunk - 1], sub
                        )
                        gpsimd.tensor_tensor(
                            gb[:, 0:1], xb[:, 0:1], xb[:, n - 1 : n], sub
                        ).then_inc(PS, 1)
                    else:
                        gpsimd.tensor_tensor(
                            gb[:], xb[:, c0 : c0 + chunk],
                            xb[:, c0 - 1 : c0 + chunk - 1], sub
                        ).then_inc(PS, 1)

        @block.vector
        def _(vector):
            vector.wait_ge(LB, l_bcast)
            for t in range(nt):
                for c in range(nch):
                    idx = t * nch + c
                    c0 = c * chunk
                    gb = G[idx % 2]
                    tu = TU[idx % 2]
                    wm = WM[idx % 2]
                    pk = PK[idx % 2]
                    vector.wait_ge(PS, idx + 1)
                    if idx >= 2:
                        vector.wait_ge(C1, idx - 1)
                    vector.tensor_tensor(
                        A[:], gb[:], THR[:, 0 * n + c0 : 0 * n + c0 + chunk], is_ge
                    )
                    vector.tensor_tensor(
                        Bb[:], gb[:], THR[:, 1 * n + c0 : 1 * n + c0 + chunk], is_le
                    )
                    vector.tensor_tensor(
                        A2[:], gb[:], THR[:, 2 * n + c0 : 2 * n + c0 + chunk], is_ge
                    )
                    vector.tensor_tensor(
                        B2[:], gb[:], THR[:, 3 * n + c0 : 3 * n + c0 + chunk], is_le
                    )
                    vector.drain()
                    tu3 = tu[:].rearrange("p (g k) -> p g k", k=16)
                    vector.tensor_tensor(
                        tu3[:, :, 0:8],
                        A[:].rearrange("p (g k) -> p g k", k=8),
                        Bb[:].rearrange("p (g k) -> p g k", k=8),
                        mul,
                    )
                    vector.tensor_tensor(
                        tu3[:, :, 8:16],
                        A2[:].rearrange("p (g k) -> p g k", k=8),
                        B2[:].rearrange("p (g k) -> p g k", k=8),
                        mul,
                    )
                    vector.drain()
                    vector.tensor_tensor(wm[:], tu[:], WB[:], mul)
                    vector.drain()
                    vector.tensor_reduce(
                        pk[:],
                        wm[:].rearrange("p (g k) -> p g k", k=16),
                        mybir.AxisListType.X,
                        mybir.AluOpType.add,
                    ).then_inc(V, 1)

        @block.scalar
        def _(scalar):
            for t in range(nt):
                for c in range(nch):
                    idx = t * nch + c
                    scalar.wait_ge(V, idx + 1)
                    if c == 0 and t >= 2:
                        scalar.wait_ge(SB[t % 2], 16 * (t // 2))
                    scalar.copy(
                        OUT[t % 2][:, c * npk_t : (c + 1) * npk_t],
                        PK[idx % 2][:],
                    ).then_inc(C1, 1)

    return nc


_STATE = {}
# Pipeline groups as core counts (same per-core NEFF for every group).
# Big group first: its output fetch + decode hide under the remaining
# q-slab uploads; the small last group minimizes the exposed tail.
GROUP_CORES = [3, 2, 2, 1]
assert sum(GROUP_CORES) == N_CORES


def _get_runner():
    """Build (once) the cached jitted shard_map executables, one per
    device group (the batch is pipelined across groups so host work
    overlaps the ~40 MB/s tunnel transfers)."""
    if "groups" in _STATE:
        return _STATE

    import jax
    import jax.numpy as jnp
    from jax.sharding import Mesh, PartitionSpec, NamedSharding
    from concourse import bass2jax

    try:
        from jax.experimental.shard_map import shard_map
    except ImportError:
        from jax.sharding import shard_map

    bass2jax.install_neuronx_cc_hook()

    nc = build_nc()
    assert nc.dbg_addr is None
    pid_name = nc.partition_id_tensor.name if nc.partition_id_tensor else None

    in_names = []
    out_names = []
    out_avals = []
    for alloc in nc.m.functions[0].allocations:
        if not isinstance(alloc, mybir.MemoryLocationSet):
            continue
        name = alloc.memorylocations[0].name
        if alloc.kind == "ExternalInput":
            if name != pid_name:
                in_names.append(name)
        elif alloc.kind == "ExternalOutput":
            out_names.append(name)
            out_avals.append(
                jax.core.ShapedArray(
                    tuple(alloc.tensor_shape), mybir.dt.np(alloc.dtype)
                )
            )
    assert in_names == ["xq", "thr", "wrow"], in_names
    assert out_names == ["out"], out_names
    all_in_names = tuple(in_names) + tuple(out_names)
    if pid_name is not None:
        all_in_names = all_in_names + (pid_name,)

    def _body(xq_a, thr_a, wrow_a, ybuf_a):
        operands = [xq_a, thr_a, wrow_a, ybuf_a]
        if pid_name is not None:
            operands.append(bass2jax.partition_id_tensor())
        outs = bass2jax._bass_exec_p.bind(
            *operands,
            out_avals=tuple(out_avals),
            in_names=all_in_names,
            out_names=tuple(out_names),
            lowering_input_output_aliases=(),
            sim_require_finite=True,
            sim_require_nnan=True,
            nc=nc,
        )
        return outs[0]

    devices = jax.devices()[:N_CORES]
    assert len(devices) == N_CORES
    p_core = PartitionSpec("core")

    import ml_dtypes

    wrow_one = np.tile((2.0 ** np.arange(16)).astype(ml_dtypes.bfloat16), 2 * CH // 16)
    groups = []
    c0 = 0
    for s, ncores in enumerate(GROUP_CORES):
        mesh = Mesh(np.asarray(devices[c0 : c0 + ncores]), ("core",))
        c0 += ncores
        sh = NamedSharding(mesh, p_core)
        rows = ncores * ROWS
        fn = jax.jit(
            shard_map(
                _body,
                mesh=mesh,
                in_specs=(p_core, p_core, p_core, p_core),
                out_specs=p_core,
                check_rep=False,
            ),
            donate_argnums=(3,),
            keep_unused=True,
        )
        zeros_fn = jax.jit(
            lambda rr=rows: jnp.zeros((rr, N // 8), jnp.uint16),
            out_shardings=sh,
        )
        groups.append(
            dict(
                fn=fn,
                zeros_fn=zeros_fn,
                sh=sh,
                rows=rows,
                ncores=ncores,
                w_dev=jax.device_put(np.tile(wrow_one, ncores), sh),
            )
        )
    _STATE.update(
        groups=groups,
        bf16=ml_dtypes.bfloat16,
        jax=jax,
        qbuf=np.empty((max(GROUP_CORES) * ROWS, N), np.float32),
        q8=np.empty((B, N), np.int8),
    )
    return _STATE


def _warmup():
    """Compile the jitted executables and run one dummy exec per group so
    the first real kernel() call pays no compile cost.  Best-effort."""
    try:
        st = _get_runner()
        outs = []
        for grp in st["groups"]:
            q_dev = st["jax"].device_put(
                np.zeros((grp["rows"], N), np.int8), grp["sh"]
            )
            thr_dev = st["jax"].device_put(
                np.zeros(grp["ncores"] * 4 * N, st["bf16"]), grp["sh"]
            )
            outs.append(grp["fn"](q_dev, thr_dev, grp["w_dev"], grp["zeros_fn"]()))
            grp["ybuf_next"] = grp["zeros_fn"]()
        for o in outs:
            np.asarray(o)
    except Exception:
        _STATE.clear()


_warmup()


def kernel(output, mean_grad, var_grad, k):
    import os
    import time as _time

    _tt = [] if os.environ.get("KBENCH") else None

    def _mark(label):
        if _tt is not None:
            _tt.append((label, _time.time()))

    st = _get_runner()
    jax = st["jax"]
    _mark("start")

    x = np.ascontiguousarray(np.asarray(output, dtype=np.float32))
    assert x.shape == (B, N), x.shape
    mg = np.asarray(mean_grad, dtype=np.float32)
    vg = np.asarray(var_grad, dtype=np.float32)
    kf = np.float32(k)

    # f32 bounds, bit-matching the reference
    std = np.sqrt(vg, dtype=np.float32)
    ks = (kf * std).astype(np.float32)
    lower = (mg - ks).astype(np.float32)
    upper = (mg + ks).astype(np.float32)

    # pipelined: quantize + upload each group's slab, dispatch all execs
    # (device_put / jit dispatch are async; only np.asarray blocks).
    # Scale + thresholds are per group, so the first upload starts after
    # scanning only the first slab; thresholds ship before the q slab so
    # the exec can start the moment q lands.
    buf, q8 = st["qbuf"], st["q8"]
    lod = lower.astype(np.float64)
    upd = upper.astype(np.float64)
    outs = []
    scales = []
    r0 = 0
    for s, grp in enumerate(st["groups"]):
        r1 = r0 + grp["rows"]
        xh = x[r0:r1]
        maxabs = max(float(xh.max()), -float(xh.min()))
        if maxabs == 0.0:
            maxabs = 1.0
        scales.append(maxabs)
        los = lod * (127.0 / maxabs)
        ups = upd * (127.0 / maxabs)
        L1 = np.clip(np.ceil(los + MARGIN), -255, 255)
        U1 = np.clip(np.floor(ups - MARGIN), -255, 255)
        L0 = np.clip(np.ceil(los - MARGIN), -255, 255)
        U0 = np.clip(np.floor(ups + MARGIN), -255, 255)
        thr_np = np.tile(
            np.concatenate([L1, U1, L0, U0]).astype(st["bf16"]), grp["ncores"]
        )
        thr_dev = jax.device_put(thr_np, grp["sh"])
        bufh = buf[: grp["rows"]]
        np.multiply(xh, np.float32(127.0 / maxabs), out=bufh)
        np.rint(bufh, out=bufh)
        np.copyto(q8[r0:r1], bufh, casting="unsafe")
        q_dev = jax.device_put(q8[r0:r1], grp["sh"])
        ybuf = grp.pop("ybuf_next", None)
        if ybuf is None:
            ybuf = grp["zeros_fn"]()
        outs.append(grp["fn"](q_dev, thr_dev, grp["w_dev"], ybuf))
        _mark(f"issued{s}")
        r0 = r1
    for o in outs:
        if hasattr(o, "copy_to_host_async"):
            o.copy_to_host_async()
    # pre-create next call's donated output buffers (off the issue path)
    for grp in st["groups"]:
        if "ybuf_next" not in grp:
            grp["ybuf_next"] = grp["zeros_fn"]()

    def _decode(pk, xh, yh):
        # low byte = tight bits, high byte = loose bits (little endian)
        rows = pk.shape[0]
        byte_view = pk.view(np.uint8).reshape(rows, N // 8, 2)
        t_bytes = np.ascontiguousarray(byte_view[:, :, 0])
        tight = np.unpackbits(t_bytes, axis=1, bitorder="little")
        np.multiply(xh, tight, out=yh)

        # exact fixup of the uncertain band (loose & ~tight), sparse decode
        u_bytes = (byte_view[:, :, 1] & ~t_bytes).ravel()
        nzb = np.flatnonzero(u_bytes)
        if nzb.size:
            bits = np.unpackbits(u_bytes[nzb], bitorder="little")
            pos = np.flatnonzero(bits)
            idx = (nzb[pos >> 3] << 3) + (pos & 7)
            xr = xh.ravel()
            cols = idx & (N - 1)
            prev = idx - 1 + ((cols == 0).astype(np.int64) << 13)
            g_ex = xr[idx] - xr[prev]
            keep = (g_ex >= lower[cols]) & (g_ex <= upper[cols])
            yh.reshape(-1)[idx] = np.where(keep, xr[idx], np.float32(0.0))

    y = np.empty_like(x)
    g0 = 0
    for s, out_dev in enumerate(outs):
        _mark(f"prefetch{s}")
        shards = getattr(out_dev, "addressable_shards", None)
        if shards is not None and len(shards) > 1:
            for shd in sorted(shards, key=lambda q: q.index[0].start or 0):
                r0 = g0 + (shd.index[0].start or 0)
                pk = np.asarray(shd.data)
                _decode(pk, x[r0 : r0 + pk.shape[0]], y[r0 : r0 + pk.shape[0]])
        else:
            pk = np.asarray(out_dev)
            _decode(pk, x[g0 : g0 + pk.shape[0]], y[g0 : g0 + pk.shape[0]])
        g0 += st["groups"][s]["rows"]
        _mark(f"post{s}")
    if _tt is not None:
        t0 = _tt[0][1]
        _STATE["last_times"] = [(l, t - t0) for l, t in _tt]
    return y
